# revision 1
# baseline (speedup 1.0000x reference)
"""DeepSeek-V2 decode layer on 8 TRN2 NeuronCores (Bass/Tile SPMD kernel).

Sharding (per core c of 8):
  - QKV proj: row-parallel (528 of 4224 used rows each), then AllToAll to
    redistribute q^T from row-sharded to batch-sharded layout.
  - Attention: data-parallel over batch (8 sequences per core); kT cache fed
    host-transposed (d-major) so scores matmul needs no on-device transpose.
  - Wo: output-column-parallel (512 cols each) after AllGather of ctx.
  - post-attn hidden: AllGather of per-core column slices -> replicated.
  - MoE: expert-parallel (expert c on core c), dense compute for all 64
    tokens, combine weights applied per-core, AllReduce sum at the end.
  - rmsnorm weights (the x64 normalization factor and attention scale) are
    folded into the weight matrices host-side; the 1/sqrt(ms) per-token
    scale is applied at the qkv/gate/w1-input stage; all weights are fed
    pre-transposed so the contraction dim lands on SBUF partitions.
Matmuls run as float32r (full PE rate at moving-dim>=256, ~fp22 precision).
DMA: weight streams ride the SP HWDGE queue, traced in consumption order so
they prefetch from t=0; latency-critical per-phase traffic rides the ACT
HWDGE queue so it is never stuck behind bulk weight transfers.
"""

import os
import sys

import numpy as np

for _p in ("/opt/trn_rl_repo", "/root/.axon_site/_ro/trn_rl_repo", "/root/.axon_site"):
    if _p not in sys.path and os.path.isdir(_p):
        sys.path.append(_p)

def _ensure_ntff_hook():
    """This image's antenv lacks axon_hooks; shim it so BASS_TRACE works."""
    import types

    try:
        import antenv.axon_hooks  # noqa: F401
        return
    except ImportError:
        pass
    import antenv

    mod = types.ModuleType("antenv.axon_hooks")
    _state = {"h": None}
    mod.set_axon_ntff_profile_hook = lambda h: _state.__setitem__("h", h)
    mod.get_axon_ntff_profile_hook = lambda: _state["h"]
    sys.modules["antenv.axon_hooks"] = mod
    antenv.axon_hooks = mod
    try:
        sys.path.insert(0, "/root/.axon_site/trn_agent_boot")
        import trn_boot

        so_path = "/opt/axon/libaxon_pjrt.so"
        if os.path.exists(so_path):
            mod.set_axon_ntff_profile_hook(
                trn_boot._ntff_profile_via_ctypes(so_path))
    except Exception as e:  # tracing degrades; compile+run still work
        print(f"ntff hook install failed: {e}")


_ensure_ntff_hook()

import concourse.bacc as bacc
import concourse.bass as bass
import concourse.mybir as mybir
import concourse.tile as tile
from concourse.bass_utils import run_bass_kernel_spmd
from concourse.masks import make_identity
from contextlib import ExitStack

F32 = mybir.dt.float32
F32R = mybir.dt.float32r
AF = mybir.ActivationFunctionType
ALU = mybir.AluOpType

B, HID, S, NH, HD = 64, 4096, 4096, 32, 128
QROWS = NH * HD + HD          # 4224 used rows of Wqkv (q + current-k)
RPC = QROWS // 8              # 528 qkv rows per core
NB = B // 8                   # 8 batches per core
MI, TWO_MI = 1408, 2816
NC_ = 8
EPS = 1e-6

LAST_RESULT = None            # BassKernelResults of the most recent run


def _r(ap):
    return ap.bitcast(F32R)


_rr = _r  # producer-side relabel: walrus requires fp32r-typed producers


def _build_program():
    nc = bacc.Bacc(None, target_bir_lowering=False, num_devices=NC_)

    hid_i = nc.dram_tensor("hid", [B, HID], F32, kind="ExternalInput")
    hidT_i = nc.dram_tensor("hidT", [128, 32 * 64], F32, kind="ExternalInput")
    hidc_i = nc.dram_tensor("hidcols", [B, 512], F32, kind="ExternalInput")
    wqkvT_i = nc.dram_tensor("wqkvT", [HID, RPC], F32, kind="ExternalInput")
    woT_i = nc.dram_tensor("woT", [HID, 512], F32, kind="ExternalInput")
    gateT_i = nc.dram_tensor("gateT", [HID, 8], F32, kind="ExternalInput")
    w1T_i = nc.dram_tensor("w1T", [HID, TWO_MI], F32, kind="ExternalInput")
    w2T_i = nc.dram_tensor("w2T", [MI, HID], F32, kind="ExternalInput")
    kT_i = nc.dram_tensor("kT", [NB, HD, S], F32, kind="ExternalInput")
    v_i = nc.dram_tensor("v", [NB, 128, 32 * HD], F32, kind="ExternalInput")
    seqm1_i = nc.dram_tensor("seqm1", [1, NB], F32, kind="ExternalInput")
    sel_i = nc.dram_tensor("sel", [1, 8], F32, kind="ExternalInput")
    iota_i = nc.dram_tensor("iota2d", [128, 32], F32, kind="ExternalInput")
    ones_i = nc.dram_tensor("ones", [128, 1], F32, kind="ExternalInput")
    out_o = nc.dram_tensor("out", [B, HID], F32, kind="ExternalOutput")

    rg = [list(range(NC_))]

    with tile.TileContext(nc) as tc, ExitStack() as top:
        dramp = top.enter_context(tc.tile_pool(name="dram", bufs=1, space="DRAM"))
        a2a_in = dramp.tile([QROWS, NB], F32)
        a2a_out = dramp.tile([QROWS, NB], F32)
        ctx_b = dramp.tile([NB, HID], F32)
        agc = dramp.tile([B, HID], F32, addr_space="Shared")
        hsl_b = dramp.tile([B, 512], F32)
        agh = dramp.tile([B * 8, 512], F32, addr_space="Shared")
        moe_b = dramp.tile([B, HID], F32)
        ar_o = dramp.tile([B, HID], F32, addr_space="Shared")

        const = top.enter_context(tc.tile_pool(name="const", bufs=1))
        ident64 = const.tile([64, 64], F32)
        make_identity(nc, ident64)
        ident128 = const.tile([128, 128], F32)
        make_identity(nc, ident128)
        ones_col = const.tile([128, 1], F32)
        nc.scalar.dma_start(_rr(ones_col[:]), _rr(ones_i[:]))
        zero_col = const.tile([128, 1], F32)
        nc.gpsimd.memset(zero_col[:], 0.0)
        eps_col = const.tile([128, 1], F32)
        nc.gpsimd.memset(eps_col[:], float(HID) * EPS)
        nc.const_aps.aps[(F32, 0.0)] = zero_col[:]
        nc.const_aps.aps[(F32, float(HID) * EPS)] = eps_col[:]
        iota_sb = const.tile([128, 32], F32)
        nc.scalar.dma_start(iota_sb[:], iota_i[:])
        sel_bc = const.tile([64, 8], F32)
        nc.scalar.dma_start(sel_bc[:], sel_i.ap().to_broadcast((64, 8)))

        # long-lived activations
        acts = top.enter_context(tc.tile_pool(name="acts", bufs=1))
        scratch = acts.tile([B, HID], F32)       # square scratch / moe / final
        hid_full = acts.tile([B, HID], F32)      # post-attn residual hidden
        xT = acts.tile([128, 32 * 64], F32)
        midT = acts.tile([128, 11 * 64], F32)
        small = top.enter_context(tc.tile_pool(name="small", bufs=1))

        # ---------------- Phase A: norm1 -> qkv -> A2A ----------------
        with ExitStack() as pa:
            sA = pa.enter_context(tc.tile_pool(name="sA", bufs=1))
            ptA = pa.enter_context(tc.tile_pool(name="ptA", bufs=2, space="PSUM"))
            qkvps = pa.enter_context(tc.tile_pool(name="qkvps", bufs=1, space="PSUM"))
            wqp = pa.enter_context(tc.tile_pool(name="wqp", bufs=3))

            hT = sA.tile([128, 32 * 64], F32)
            nc.scalar.dma_start(_rr(hT[:]), _rr(hidT_i[:]))
            h_sb = sA.tile([B, HID], F32)
            nc.scalar.dma_start(h_sb[:], hid_i[:])
            ssq = small.tile([64, 1], F32, name="ssq")
            nc.scalar.activation(scratch[:], h_sb[:], AF.Square, accum_out=ssq[:])
            rs_col = small.tile([64, 1], F32, name="rs_col")
            nc.scalar.activation(rs_col[:], ssq[:], AF.Sqrt, bias=float(HID) * EPS)
            nc.vector.reciprocal(rs_col[:], rs_col[:])

            q1 = qkvps.tile([64, 512], F32, name="q1")
            q2 = qkvps.tile([64, 16], F32, name="q2")
            for k in range(32):
                wq = wqp.tile([128, RPC], F32, name="wq", tag="wq")
                nc.sync.dma_start(_rr(wq[:]), _rr(wqkvT_i[k * 128:(k + 1) * 128, :]))
                nc.tensor.matmul(q1[:], _r(hT[:, k * 64:(k + 1) * 64]),
                                 _r(wq[:, :512]), start=(k == 0), stop=(k == 31))
                nc.tensor.matmul(q2[:], _r(hT[:, k * 64:(k + 1) * 64]),
                                 _r(wq[:, 512:RPC]), start=(k == 0), stop=(k == 31))
            qkv_sb = sA.tile([64, RPC], F32)
            nc.vector.tensor_scalar_mul(qkv_sb[:, :512], q1[:], rs_col[:])
            nc.vector.tensor_scalar_mul(qkv_sb[:, 512:RPC], q2[:], rs_col[:])

            # transpose (64, 528) -> chunks of (128, 64), scatter into A2A input
            # a2a_in flat block s (rows 528s..528s+528) = qkv^T[:, 8s:8s+8]
            a2a_view = a2a_in.rearrange("(s q) j -> q s j", s=8)  # (528, 8, 8)
            for jt in range(5):
                rows = 128 if jt < 4 else 16
                pt = ptA.tile([128, 64], F32, name="ptA_q", tag="ptA_t")
                nc.tensor.transpose(pt[:rows, :],
                                    qkv_sb[:, jt * 128: jt * 128 + rows], ident64[:])
                qs = sA.tile([128, 64], F32, name="qs", tag="qs", bufs=2)
                nc.vector.tensor_copy(qs[:rows, :], pt[:rows, :])
                src = qs[:rows, :].rearrange("p (s j) -> p s j", s=8)
                nc.scalar.dma_start(a2a_view[jt * 128: jt * 128 + rows], src)

            nc.gpsimd.collective_compute(
                "AllToAll", ALU.bypass, replica_groups=rg,
                ins=[a2a_in.opt()], outs=[a2a_out.opt()],
            )

        # ------- weight streams (SP queue, traced in consumption order) -------
        wop = top.enter_context(tc.tile_pool(name="wop", bufs=3))
        gwp = top.enter_context(tc.tile_pool(name="gwp", bufs=2))
        w1p = top.enter_context(tc.tile_pool(name="w1p", bufs=4))
        wo_tiles, gw_tiles, w1_tiles = [], [], []
        for k in range(32):
            wo_t = wop.tile([128, 512], F32, name="wo_t", tag="wo")
            nc.sync.dma_start(_rr(wo_t[:]), _rr(woT_i[k * 128:(k + 1) * 128, :]))
            wo_tiles.append(wo_t)
        for k in range(32):
            gw = gwp.tile([128, 8], F32, name="gw", tag="gw")
            nc.sync.dma_start(_rr(gw[:]), _rr(gateT_i[k * 128:(k + 1) * 128, :]))
            gw_tiles.append(gw)
        for k in range(32):
            w1t = w1p.tile([128, TWO_MI], F32, name="w1t", tag="w1")
            for hh in range(2):
                fs = slice(hh * MI, (hh + 1) * MI)
                nc.sync.dma_start(_rr(w1t[:, fs]),
                                  _rr(w1T_i[k * 128:(k + 1) * 128, fs]))
            w1_tiles.append(w1t)

        # ---------------- Phase B: attention (8 local batches) ----------------
        with ExitStack() as pb:
            sB = pb.enter_context(tc.tile_pool(name="sB", bufs=1))
            kvp = pb.enter_context(tc.tile_pool(name="kvp", bufs=2))
            ppp = pb.enter_context(tc.tile_pool(name="ppp", bufs=2))
            smb = pb.enter_context(tc.tile_pool(name="smb", bufs=2))
            drb = pb.enter_context(tc.tile_pool(name="drb", bufs=2, space="DRAM"))
            scps = pb.enter_context(tc.tile_pool(name="scps", bufs=2, space="PSUM"))
            dps = pb.enter_context(tc.tile_pool(name="dps", bufs=2, space="PSUM"))
            ctxps = pb.enter_context(tc.tile_pool(name="ctxps", bufs=2, space="PSUM"))
            ctnps = pb.enter_context(tc.tile_pool(name="ctnps", bufs=2, space="PSUM"))

            # q^T for all local batches: (128, kk=33, j=8); row kk*128+p of A2A out
            qT = sB.tile([128, 33, NB], F32)
            nc.scalar.dma_start(_rr(qT[:]),
                                _rr(a2a_out.rearrange("(kk p) j -> p kk j", p=128)))

            for j in range(NB):
                kT_sb = kvp.tile([128, S], F32, name="kT_sb", tag="kT")
                for hh in range(4):
                    sl = slice(hh * (S // 4), (hh + 1) * (S // 4))
                    nc.scalar.dma_start(_rr(kT_sb[:, sl]), _rr(kT_i[j][:, sl]))
                v_sb = kvp.tile([128, 32, HD], F32, name="v_sb", tag="v")
                vv = v_i[j].rearrange("p (c d) -> p c d", d=HD)
                for hh in range(4):
                    sl = slice(hh * 8, (hh + 1) * 8)
                    nc.scalar.dma_start(_rr(v_sb[:, sl, :]), _rr(vv[:, sl, :]))
                sv_col = smb.tile([128, 1], F32, name="sv_col", tag="sv")
                nc.scalar.dma_start(sv_col[:],
                                    seqm1_i[0:1, j:j + 1].to_broadcast((128, 1)))
                vcur = smb.tile([1, HD], F32, name="vcur", tag="vcur")
                nc.scalar.dma_start(
                    _rr(vcur[:]),
                    _rr(a2a_out.rearrange("q j -> j q")[j:j + 1, 4096:4224]))

                qT_b = qT[:, 0:32, j]          # (128, 32) strided: q^T for batch
                s0 = scps.tile([128, 512], F32, name="s0", tag="sc")
                s1 = scps.tile([128, 512], F32, name="s1", tag="sc")
                for c in range(32):
                    dst = (s0 if c < 16 else s1)[:, (c % 16) * 32:(c % 16) * 32 + 32]
                    nc.tensor.matmul(dst, _r(kT_sb[:, c * 128:(c + 1) * 128]),
                                     _r(qT_b), start=True, stop=True)
                cur = dps.tile([1, 32], F32, name="cur", tag="dsm")
                nc.tensor.matmul(cur[:], _r(qT[:, 32, j:j + 1]), _r(qT_b),
                                 start=True, stop=True)

                pp = ppp.tile([128, 1024], F32, name="pp", tag="pp")
                nc.scalar.activation(_rr(pp[:, :512]), s0[:], AF.Exp)
                nc.scalar.activation(_rr(pp[:, 512:]), s1[:], AF.Exp)
                pcur = smb.tile([1, 32], F32, name="pcur", tag="pcur")
                nc.scalar.activation(_rr(pcur[:]), cur[:], AF.Exp)

                m0 = smb.tile([128, 16], F32, name="m0", tag="m0")
                m1 = smb.tile([128, 16], F32, name="m1", tag="m1")
                nc.vector.tensor_scalar(m0[:], iota_sb[:, 0:16], sv_col[:], None,
                                        op0=ALU.is_lt)
                nc.vector.tensor_scalar(m1[:], iota_sb[:, 16:32], sv_col[:], None,
                                        op0=ALU.is_lt)
                pp3 = pp[:].rearrange("p (c h) -> p c h", h=32)
                nc.vector.tensor_tensor(
                    _rr(pp3[:, 0:16]), pp3[:, 0:16],
                    m0[:, :, None].to_broadcast((128, 16, 32)), op=ALU.mult)
                nc.vector.tensor_tensor(
                    _rr(pp3[:, 16:32]), pp3[:, 16:32],
                    m1[:, :, None].to_broadcast((128, 16, 32)), op=ALU.mult)

                dsum = dps.tile([1, 512], F32, name="dsum", tag="dsm")
                nc.tensor.matmul(dsum[:], _r(ones_col[:]), _r(pp[:, :512]),
                                 start=True, stop=False)
                nc.tensor.matmul(dsum[:], _r(ones_col[:]), _r(pp[:, 512:]),
                                 start=False, stop=True)
                den = smb.tile([1, 32], F32, name="den", tag="den")
                nc.vector.reduce_sum(den[:],
                                     dsum[:].rearrange("p (c h) -> p h c", h=32),
                                     axis=mybir.AxisListType.X)
                nc.vector.tensor_tensor(den[:], den[:], pcur[:], op=ALU.add)
                rden = smb.tile([1, 32], F32, name="rden", tag="rden")
                nc.vector.reciprocal(rden[:], den[:])
                rd_d = drb.tile([1, 32], F32, name="rd_d", tag="rd")
                nc.scalar.dma_start(rd_d[:], rden[:])
                rden_bc = smb.tile([128, 32], F32, name="rden_bc", tag="rdbc")
                nc.scalar.dma_start(rden_bc[:], rd_d.to_broadcast((128, 32)))

                ctx = ctxps.tile([128, 32], F32, name="ctx", tag="ctx")
                for c in range(32):
                    nc.tensor.matmul(ctx[:], _r(v_sb[:, c, :]),
                                     _r(pp[:, c * 32:(c + 1) * 32]),
                                     start=(c == 0), stop=False)
                nc.tensor.matmul(ctx[:], _r(vcur[:]), _r(pcur[:]),
                                 start=False, stop=True)
                ctxT_sb = smb.tile([128, 32], F32, name="ctxT_sb", tag="ctxs")
                nc.vector.tensor_tensor(ctxT_sb[:], ctx[:], rden_bc[:], op=ALU.mult)
                ctn = ctnps.tile([32, 128], F32, name="ctn", tag="ctn")
                nc.tensor.transpose(ctn[:], ctxT_sb[:], ident128[:])
                ctn_sb = smb.tile([32, 128], F32, name="ctn_sb", tag="ctns")
                nc.vector.tensor_copy(ctn_sb[:], ctn[:])
                nc.scalar.dma_start(
                    ctx_b[j:j + 1, :].rearrange("o (h d) -> h (o d)", d=HD),
                    ctn_sb[:])

        # ---------------- Phase C: AG ctx -> Wo -> residual -> AG hidden ------
        with ExitStack() as pc:
            sC = pc.enter_context(tc.tile_pool(name="sC", bufs=1))
            wops = pc.enter_context(tc.tile_pool(name="wops", bufs=1, space="PSUM"))
            ptC = pc.enter_context(tc.tile_pool(name="ptC", bufs=2, space="PSUM"))

            nc.gpsimd.collective_compute(
                "AllGather", ALU.bypass, replica_groups=rg,
                ins=[ctx_b.opt()], outs=[agc.opt()],
            )
            ctx_all = sC.tile([B, HID], F32)
            nc.scalar.dma_start(ctx_all[:], agc[:])
            ctxA = sC.tile([128, 32 * 64], F32)
            for k in range(32):
                pt = ptC.tile([128, 64], F32, name="ptC_t", tag="ptC_t")
                nc.tensor.transpose(pt[:], ctx_all[:, k * 128:(k + 1) * 128],
                                    ident64[:])
                nc.vector.tensor_copy(_rr(ctxA[:, k * 64:(k + 1) * 64]), _rr(pt[:]))

            wo_ps = wops.tile([64, 512], F32)
            for k in range(32):
                nc.tensor.matmul(wo_ps[:], _r(ctxA[:, k * 64:(k + 1) * 64]),
                                 _r(wo_tiles[k][:]),
                                 start=(k == 0), stop=(k == 31))
            hidc = sC.tile([64, 512], F32)
            nc.scalar.dma_start(hidc[:], hidc_i[:])
            hsl = sC.tile([64, 512], F32)
            nc.vector.tensor_tensor(hsl[:], wo_ps[:], hidc[:], op=ALU.add)
            nc.scalar.dma_start(hsl_b[:], hsl[:])
            nc.gpsimd.collective_compute(
                "AllGather", ALU.bypass, replica_groups=rg,
                ins=[hsl_b.opt()], outs=[agh.opt()],
            )
            nc.scalar.dma_start(hid_full[:].rearrange("b (r o) -> b r o", r=8),
                                agh.rearrange("(r b) o -> b r o", b=64))

        # ---------------- Phase D: norm2 -> x^T -> gate -> top2 ----------------
        wsel_col = small.tile([64, 1], F32, name="wsel_col")
        with ExitStack() as pd:
            sD = pd.enter_context(tc.tile_pool(name="sD", bufs=1))
            ptD = pd.enter_context(tc.tile_pool(name="ptD", bufs=2, space="PSUM"))
            gps = pd.enter_context(tc.tile_pool(name="gps", bufs=1, space="PSUM"))

            ssq2 = small.tile([64, 1], F32, name="ssq2")
            nc.scalar.activation(scratch[:], hid_full[:], AF.Square,
                                 accum_out=ssq2[:])
            rs2 = small.tile([64, 1], F32, name="rs2")
            nc.scalar.activation(rs2[:], ssq2[:], AF.Sqrt, bias=float(HID) * EPS)
            nc.vector.reciprocal(rs2[:], rs2[:])
            x_sb = sD.tile([B, HID], F32)
            nc.vector.tensor_scalar_mul(x_sb[:], hid_full[:], rs2[:])

            for k in range(32):
                pt = ptD.tile([128, 64], F32, name="ptD_t", tag="ptD_t")
                nc.tensor.transpose(pt[:], x_sb[:, k * 128:(k + 1) * 128], ident64[:])
                nc.vector.tensor_copy(_rr(xT[:, k * 64:(k + 1) * 64]), _rr(pt[:]))

            g_ps = gps.tile([64, 8], F32)
            for k in range(32):
                nc.tensor.matmul(g_ps[:], _r(xT[:, k * 64:(k + 1) * 64]),
                                 _r(gw_tiles[k][:]),
                                 start=(k == 0), stop=(k == 31))
            pg = sD.tile([64, 8], F32)
            nc.scalar.activation(pg[:], g_ps[:], AF.Exp)
            m1c = sD.tile([64, 1], F32)
            nc.vector.reduce_max(m1c[:], pg[:], axis=mybir.AxisListType.X)
            eq1 = sD.tile([64, 8], F32)
            nc.vector.tensor_scalar(eq1[:], pg[:], m1c[:], None, op0=ALU.is_ge)
            t1 = sD.tile([64, 8], F32)
            nc.vector.tensor_tensor(t1[:], pg[:], eq1[:], op=ALU.mult)
            nc.vector.tensor_tensor(t1[:], pg[:], t1[:], op=ALU.subtract)
            m2c = sD.tile([64, 1], F32)
            nc.vector.reduce_max(m2c[:], t1[:], axis=mybir.AxisListType.X)
            keep = sD.tile([64, 8], F32)
            nc.vector.tensor_scalar(keep[:], pg[:], m2c[:], None, op0=ALU.is_ge)
            wsum = sD.tile([64, 1], F32)
            nc.vector.tensor_tensor(wsum[:], m1c[:], m2c[:], op=ALU.add)
            nc.vector.reciprocal(wsum[:], wsum[:])
            wts = sD.tile([64, 8], F32)
            nc.vector.tensor_tensor(wts[:], pg[:], keep[:], op=ALU.mult)
            nc.vector.tensor_scalar_mul(wts[:], wts[:], wsum[:])
            nc.vector.tensor_tensor(wts[:], wts[:], sel_bc[:], op=ALU.mult)
            nc.vector.reduce_sum(wsel_col[:], wts[:], axis=mybir.AxisListType.X)

        # ---------------- Phase E: MoE expert FFN + AllReduce ----------------
        with ExitStack() as pe1:
            gups = pe1.enter_context(tc.tile_pool(name="gups", bufs=1, space="PSUM"))
            ptE = pe1.enter_context(tc.tile_pool(name="ptE", bufs=2, space="PSUM"))
            sE = pe1.enter_context(tc.tile_pool(name="sE", bufs=1))

            gu = gups.tile([64, TWO_MI], F32)
            slices = [(o * 512, min(512, TWO_MI - o * 512)) for o in range(6)]
            for k in range(32):
                w1t = w1_tiles[k]
                for (off, w) in slices:
                    nc.tensor.matmul(gu[:, off:off + w],
                                     _r(xT[:, k * 64:(k + 1) * 64]),
                                     _r(w1t[:, off:off + w]),
                                     start=(k == 0), stop=(k == 31))
            sg = sE.tile([64, MI], F32)
            nc.scalar.activation(sg[:], gu[:, :MI], AF.Silu)
            mid = sE.tile([64, MI], F32)
            nc.vector.tensor_tensor(mid[:], sg[:], gu[:, MI:], op=ALU.mult)

            for mk in range(11):
                pt = ptE.tile([128, 64], F32, name="ptE_t", tag="ptE_t")
                nc.tensor.transpose(pt[:], mid[:, mk * 128:(mk + 1) * 128],
                                    ident64[:])
                nc.vector.tensor_copy(_rr(midT[:, mk * 64:(mk + 1) * 64]), _rr(pt[:]))

        with ExitStack() as pe2:
            w2p = pe2.enter_context(tc.tile_pool(name="w2p", bufs=3))
            mops = pe2.enter_context(tc.tile_pool(name="mops", bufs=2, space="PSUM"))
            sF = pe2.enter_context(tc.tile_pool(name="sF", bufs=1))
            w2_tiles = []
            for mk in range(11):
                w2t = w2p.tile([128, HID], F32, name="w2t", tag="w2")
                for hh in range(2):
                    fs = slice(hh * 2048, (hh + 1) * 2048)
                    nc.sync.dma_start(_rr(w2t[:, fs]),
                                      _rr(w2T_i[mk * 128:(mk + 1) * 128, fs]))
                w2_tiles.append(w2t)
            mo0 = mops.tile([64, 2048], F32, name="mo0", tag="mo")
            mo1 = mops.tile([64, 2048], F32, name="mo1", tag="mo")
            for mk in range(11):
                w2t = w2_tiles[mk]
                for oh, mo in ((0, mo0), (1, mo1)):
                    for oc in range(4):
                        off = oh * 2048 + oc * 512
                        nc.tensor.matmul(mo[:, oc * 512:(oc + 1) * 512],
                                         _r(midT[:, mk * 64:(mk + 1) * 64]),
                                         _r(w2t[:, off:off + 512]),
                                         start=(mk == 0), stop=(mk == 10))
            nc.vector.tensor_scalar_mul(scratch[:, :2048], mo0[:], wsel_col[:])
            nc.vector.tensor_scalar_mul(scratch[:, 2048:], mo1[:], wsel_col[:])

            nc.scalar.dma_start(moe_b[:], scratch[:])
            nc.gpsimd.collective_compute(
                "AllReduce", ALU.add, replica_groups=rg,
                ins=[moe_b.opt()], outs=[ar_o.opt()],
            )
            ar_sb = sF.tile([B, HID], F32)
            nc.scalar.dma_start(ar_sb[:], ar_o[:])
            nc.vector.tensor_tensor(scratch[:], ar_sb[:], hid_full[:], op=ALU.add)
            nc.scalar.dma_start(out_o[:], scratch[:])

    nc.compile()
    return nc


_NC_CACHE = None


def _get_program():
    global _NC_CACHE
    if _NC_CACHE is None:
        _NC_CACHE = _build_program()
    return _NC_CACHE


def kernel(hidden_states, positions, k_cache, v_cache, seq_lens,
           norm1_w, norm2_w, Wqkv, Wo, gate_w, w1, w2):
    global LAST_RESULT
    nc = _get_program()

    hs = np.asarray(hidden_states, np.float32).reshape(B, HID)
    scale = np.float32(HD) ** -0.5
    n1 = (np.asarray(norm1_w, np.float32) * 64.0)
    n2 = (np.asarray(norm2_w, np.float32) * 64.0)

    wq = np.asarray(Wqkv, np.float32)[:QROWS] * n1[None, :]
    wq[:NH * HD] *= scale
    gT = np.ascontiguousarray((np.asarray(gate_w, np.float32) * n2[None, :]).T)
    iota2d = (np.arange(128, dtype=np.float32)[:, None]
              + 128.0 * np.arange(32, dtype=np.float32)[None, :])
    seqm1 = (np.asarray(seq_lens, np.int32).astype(np.float32) - 1.0)
    # hidT[p, k*64+b] = hs[b, 128k+p]
    hidT = np.ascontiguousarray(
        hs.T.reshape(32, 128, 64).transpose(1, 0, 2).reshape(128, 32 * 64))

    in_maps = []
    for c in range(NC_):
        bs = slice(c * NB, (c + 1) * NB)
        sel = np.zeros((1, 8), np.float32)
        sel[0, c] = 1.0
        in_maps.append({
            "hid": hs,
            "hidT": hidT,
            "hidcols": np.ascontiguousarray(hs[:, c * 512:(c + 1) * 512]),
            "wqkvT": np.ascontiguousarray(wq[c * RPC:(c + 1) * RPC].T),
            "woT": np.ascontiguousarray(
                np.asarray(Wo, np.float32)[c * 512:(c + 1) * 512].T),
            "gateT": gT,
            "w1T": np.ascontiguousarray((np.asarray(w1, np.float32)[c]
                                         * n2[None, :]).T),
            "w2T": np.ascontiguousarray(np.asarray(w2, np.float32)[c].T),
            "kT": np.ascontiguousarray(
                np.asarray(k_cache, np.float32)[bs].transpose(0, 2, 1)),
            "v": np.ascontiguousarray(np.asarray(v_cache, np.float32)[bs]
                                      .reshape(NB, 32, 128, HD)
                                      .transpose(0, 2, 1, 3)
                                      .reshape(NB, 128, 32 * HD)),
            "seqm1": np.ascontiguousarray(seqm1[bs].reshape(1, NB)),
            "sel": sel,
            "iota2d": iota2d,
            "ones": np.ones((128, 1), np.float32),
        })

    LAST_RESULT = run_bass_kernel_spmd(nc, in_maps, core_ids=list(range(NC_)))
    return LAST_RESULT.results[0]["out"].reshape(B, 1, HID).astype(np.float32)



# revision 17
# speedup vs baseline: 1.0689x; 1.0689x over previous
"""DeepSeek-V2 decode layer on 8 TRN2 NeuronCores (Bass/Tile SPMD kernel).

v2 design (bf16 + seq-truncated attention + balanced batch placement):
  - All matmul operands bf16 (fp32 PSUM accumulate); residual stream, softmax
    denominators, and gate top-2 stay fp32.  bf16-everything sim rel-err vs
    the fp32 reference is ~2.4e-3 (budget 2e-2).
  - QKV proj row-parallel (512 q rows + 16 current-k rows per core), AllToAll
    (bf16) redistributes q^T/kcur^T to batch-sharded layout.
  - Attention data-parallel: 8 sequences per core, chosen by LPT bin-packing
    on ceil((seq_len-1)/128) so all cores get the same per-slot chunk budget
    C_j (required for SPMD) with minimal padding.  Host zero-pads K^T columns
    and V rows outside [0, seq_len-1); V carries a ones-column so the softmax
    denominator falls out of the ctx matmul for free.  The current token's
    k==v vector is applied via tiny rank-1 matmuls (uniform across cores).
  - ctx matmul uses p-chunks as stationary so ctx lands directly as
    (head, dim) -- no per-batch transpose, no DRAM broadcast round-trip.
  - Wo output-column-parallel (512 cols/core) after bf16 AllGather of ctx;
    post-attn hidden AllGather in fp32 (residual precision).
  - MoE expert-parallel (1 expert/core), norm2 weights folded into w1/gate
    host-side and the 1/rms per-token scale applied on gu -- so w1 matmuls
    start right after the hidden AllGather without waiting on the rsqrt.
    Expert outputs combined via bf16 AllReduce.
  - Bulk weight DMA spread across the SP (w1) and Pool (wo, gate, w2) HWDGE
    queues; latency-critical per-phase traffic rides ACT and DVE queues.
"""

import os
import sys

import numpy as np

for _p in ("/opt/trn_rl_repo", "/root/.axon_site/_ro/trn_rl_repo", "/root/.axon_site"):
    if _p not in sys.path and os.path.isdir(_p):
        sys.path.append(_p)


def _ensure_ntff_hook():
    """This image's antenv lacks axon_hooks; shim it so BASS_TRACE works."""
    import types

    try:
        import antenv.axon_hooks  # noqa: F401
        return
    except ImportError:
        pass
    import antenv

    mod = types.ModuleType("antenv.axon_hooks")
    _state = {"h": None}
    mod.set_axon_ntff_profile_hook = lambda h: _state.__setitem__("h", h)
    mod.get_axon_ntff_profile_hook = lambda: _state["h"]
    sys.modules["antenv.axon_hooks"] = mod
    antenv.axon_hooks = mod
    try:
        sys.path.insert(0, "/root/.axon_site/trn_agent_boot")
        import trn_boot

        so_path = "/opt/axon/libaxon_pjrt.so"
        if os.path.exists(so_path):
            mod.set_axon_ntff_profile_hook(
                trn_boot._ntff_profile_via_ctypes(so_path))
    except Exception as e:  # tracing degrades; compile+run still work
        print(f"ntff hook install failed: {e}")


_ensure_ntff_hook()

import ml_dtypes
import concourse.bacc as bacc
import concourse.bass as bass
import concourse.mybir as mybir
import concourse.tile as tile
from concourse.bass_utils import run_bass_kernel_spmd
from concourse.masks import make_identity
from contextlib import ExitStack

F32 = mybir.dt.float32
BF16 = mybir.dt.bfloat16
AF = mybir.ActivationFunctionType
ALU = mybir.AluOpType
BF = ml_dtypes.bfloat16

B, HID, S, NH, HD = 64, 4096, 4096, 32, 128
MI, TWO_MI = 1408, 2816
NC_ = 8
NB = B // NC_                 # 8 local batches (slots) per core
QR_Q, QR_K = 512, 16          # per-core q rows / current-k rows of Wqkv
RPC = QR_Q + QR_K             # 528
EPS = 1e-6

LAST_RESULT = None


def _plan(seq_lens):
    """Slot budgets + batch->core placement balanced on KV chunk count.

    Real attended positions per batch are [0, sl-1) (the current token is
    handled separately), so cb = ceil((sl-1)/128).  Sort desc, group ranks
    [8j, 8j+8) into slot j with budget C[j] = group max; core c takes the
    c-th member of each group.  perm[c*8+j] = global batch index.
    """
    sl = np.asarray(seq_lens, np.int64)
    cb = np.maximum(sl - 1, 0)
    cb = -(-cb // 128)
    order = np.argsort(-cb, kind="stable")
    C = [int(cb[order[j * 8]]) for j in range(NB)]
    perm = np.empty(B, np.int64)
    for j in range(NB):
        for c in range(NC_):
            perm[c * NB + j] = order[j * 8 + c]
    return perm, tuple(C)


def _build_program(C):
    nc = bacc.Bacc(None, target_bir_lowering=False, num_devices=NC_)

    CSUM = sum(C)
    koff = np.concatenate([[0], np.cumsum(C)]).astype(int)  # chunk offsets

    hid_i = nc.dram_tensor("hid", [B, HID], F32, kind="ExternalInput")
    hidT_i = nc.dram_tensor("hidT", [128, 32 * 64], BF16, kind="ExternalInput")
    hidc_i = nc.dram_tensor("hidcols", [B, 512], F32, kind="ExternalInput")
    wqkvT_i = nc.dram_tensor("wqkvT", [128, 32, RPC], BF16, kind="ExternalInput")
    woT_i = nc.dram_tensor("woT", [128, 32, 512], BF16, kind="ExternalInput")
    gateT_i = nc.dram_tensor("gateT", [128, 32, 8], BF16, kind="ExternalInput")
    w1T_i = nc.dram_tensor("w1T", [HID, TWO_MI], BF16, kind="ExternalInput")
    w2T_i = nc.dram_tensor("w2T", [MI, HID], BF16, kind="ExternalInput")
    kT_i = nc.dram_tensor("kT", [128, max(CSUM, 1) * 128], BF16,
                          kind="ExternalInput")
    v_i = nc.dram_tensor("v", [128, max(CSUM, 1) * 129], BF16,
                         kind="ExternalInput")
    sel_i = nc.dram_tensor("sel", [1, 8], F32, kind="ExternalInput")
    out_o = nc.dram_tensor("out", [B, HID], F32, kind="ExternalOutput")

    rg = [list(range(NC_))]

    with tile.TileContext(nc) as tc, ExitStack() as top:
        # A2A block layout per dest core: [q-part (p,j,q) 128*8*4 | k-part
        # (j,t) 8*16] so the consumer-side reads are plain 3-D DMAs.
        dramp = top.enter_context(tc.tile_pool(name="dram", bufs=1, space="DRAM"))
        a2a_in = dramp.tile([NC_, RPC * NB], BF16)
        a2a_out = dramp.tile([NC_, RPC * NB], BF16)
        ctx_b = dramp.tile([NB, HID], BF16)
        agc = dramp.tile([B, HID], BF16, addr_space="Shared")
        hsl_b = dramp.tile([B, 512], F32)
        agh = dramp.tile([B * 8, 512], F32, addr_space="Shared")
        moe_b = dramp.tile([B, HID], BF16)
        ar_o = dramp.tile([B, HID], BF16, addr_space="Shared")

        const = top.enter_context(tc.tile_pool(name="const", bufs=1))
        id64b = const.tile([64, 64], BF16)
        make_identity(nc, id64b)
        id64f = const.tile([64, 64], F32)
        make_identity(nc, id64f)
        zero_col = const.tile([128, 1], F32)
        nc.gpsimd.memset(zero_col[:], 0.0)
        eps_col = const.tile([128, 1], F32)
        nc.gpsimd.memset(eps_col[:], float(HID) * EPS)
        nc.const_aps.aps[(F32, 0.0)] = zero_col[:]
        nc.const_aps.aps[(F32, float(HID) * EPS)] = eps_col[:]
        sel_bc = const.tile([64, 8], F32)
        nc.gpsimd.dma_start(sel_bc[:], sel_i.ap().to_broadcast((64, 8)))

        # long-lived activations
        acts = top.enter_context(tc.tile_pool(name="acts", bufs=1))
        scratch = acts.tile([B, HID], F32)       # square scratch / final out
        hid_full = acts.tile([B, HID], F32)      # post-attn residual hidden
        xT = acts.tile([128, 32 * 64], BF16)     # post-attn hidden transposed
        midT = acts.tile([128, 11 * 64], BF16)
        small = top.enter_context(tc.tile_pool(name="small", bufs=1))

        # ---------------- Phase A: norm1 -> qkv -> A2A ----------------
        with ExitStack() as pa:
            sA = pa.enter_context(tc.tile_pool(name="sA", bufs=1))
            ptA = pa.enter_context(tc.tile_pool(name="ptA", bufs=2, space="PSUM"))
            qkvps = pa.enter_context(tc.tile_pool(name="qkvps", bufs=1, space="PSUM"))

            hT = sA.tile([128, 32 * 64], BF16)
            nc.scalar.dma_start(hT[:], hidT_i[:])
            h_sb = sA.tile([B, HID], F32)
            nc.scalar.dma_start(h_sb[:], hid_i[:])
            wq = sA.tile([128, 32, RPC], BF16)
            nc.scalar.dma_start(wq[:, :16], wqkvT_i[:, :16])
            nc.sync.dma_start(wq[:, 16:], wqkvT_i[:, 16:])

            ssq = small.tile([64, 1], F32, name="ssq")
            nc.scalar.activation(scratch[:], h_sb[:], AF.Square, accum_out=ssq[:])
            rs_col = small.tile([64, 1], F32, name="rs_col")
            nc.scalar.activation(rs_col[:], ssq[:], AF.Sqrt, bias=float(HID) * EPS)
            nc.vector.reciprocal(rs_col[:], rs_col[:])

            q1 = qkvps.tile([64, 512], F32, name="q1")
            q2 = qkvps.tile([64, 16], F32, name="q2")
            for k in range(32):
                nc.tensor.matmul(q1[:], hT[:, k * 64:(k + 1) * 64],
                                 wq[:, k, :512], start=(k == 0), stop=(k == 31))
                nc.tensor.matmul(q2[:], hT[:, k * 64:(k + 1) * 64],
                                 wq[:, k, 512:RPC], start=(k == 0), stop=(k == 31))
            qkv_sb = sA.tile([64, RPC], BF16)
            nc.vector.tensor_scalar_mul(qkv_sb[:, :512], q1[:], rs_col[:])
            nc.vector.tensor_scalar_mul(qkv_sb[:, 512:RPC], q2[:], rs_col[:])

            # transpose (64, 528) -> chunks of (128, 64), scatter into A2A input
            a2a_q = a2a_in[:, :4096].rearrange("s (p j q) -> s p j q",
                                               p=128, j=NB)
            a2a_k = (a2a_in[:, 4096:]
                     .rearrange("s (j t) -> s j t", t=QR_K))
            for jt in range(4):
                pt = ptA.tile([128, 64], BF16, name="ptA_q", tag="ptA_t")
                nc.tensor.transpose(pt[:],
                                    qkv_sb[:, jt * 128:(jt + 1) * 128], id64b[:])
                qs = sA.tile([128, 64], BF16, name="qs", tag="qs", bufs=2)
                nc.vector.tensor_copy(qs[:], pt[:])
                for s in range(NC_):
                    nc.scalar.dma_start(
                        a2a_q[s, :, :, jt], qs[:, s * 8:(s + 1) * 8])
            for s in range(NC_):
                nc.scalar.dma_start(
                    a2a_k[s], qkv_sb[s * 8:(s + 1) * 8, 512:RPC])

            nc.gpsimd.collective_compute(
                "AllToAll", ALU.bypass, replica_groups=rg,
                ins=[a2a_in.opt()], outs=[a2a_out.opt()],
            )

        # ------- bulk weight streams (SP + Pool queues, consumption order) ----
        w1p = top.enter_context(tc.tile_pool(name="w1p", bufs=5))
        w1_tiles = []
        for k in range(32):
            w1t = w1p.tile([128, TWO_MI], BF16, name="w1t", tag="w1")
            nc.sync.dma_start(w1t[:], w1T_i[k * 128:(k + 1) * 128, :])
            w1_tiles.append(w1t)
        wop = top.enter_context(tc.tile_pool(name="wop", bufs=1))
        wo_t = wop.tile([128, 32, 512], BF16)
        nc.gpsimd.dma_start(wo_t[:], woT_i[:])
        gwp = top.enter_context(tc.tile_pool(name="gwp", bufs=1))
        gw_t = gwp.tile([128, 32, 8], BF16)
        nc.gpsimd.dma_start(gw_t[:], gateT_i[:])
        # ---------------- Phase B: attention (8 slots) ----------------
        with ExitStack() as pb:
            sB = pb.enter_context(tc.tile_pool(name="sB", bufs=1))
            kvp = pb.enter_context(tc.tile_pool(name="kvp", bufs=2))
            ppp = pb.enter_context(tc.tile_pool(name="ppp", bufs=2))
            smb = pb.enter_context(tc.tile_pool(name="smb", bufs=2))
            scps = pb.enter_context(tc.tile_pool(name="scps", bufs=2, space="PSUM"))
            curps = pb.enter_context(tc.tile_pool(name="curps", bufs=2, space="PSUM"))
            ctxps = pb.enter_context(tc.tile_pool(name="ctxps", bufs=2, space="PSUM"))

            # qT[p, j, kk]: head-dim p of q for slot j, head-block kk
            qT = sB.tile([128, NB, 32], BF16)
            kcurT = sB.tile([128, NB], BF16)
            vcur = sB.tile([1, NB * 129], BF16)
            for r in range(NC_):
                nc.scalar.dma_start(
                    qT[:, :, r * 4:(r + 1) * 4],
                    a2a_out[r, :4096].rearrange("(p j q) -> p j q", p=128, j=NB))
                kblk = a2a_out[r, 4096:].rearrange("(j t) -> j t", t=QR_K)
                nc.scalar.dma_start(
                    kcurT[r * QR_K:(r + 1) * QR_K, :],
                    kblk.rearrange("j t -> t j"))
                nc.scalar.dma_start(
                    vcur[:].rearrange("o (j x) -> o j x", x=129)
                    [:, :, r * QR_K:(r + 1) * QR_K],
                    kblk)
            nc.gpsimd.memset(
                vcur[:].rearrange("o (j x) -> o j x", x=129)[:, :, 128:129], 1.0)

            for j in range(NB):
                Cj = C[j]
                qT_b = qT[:, j, :]
                if Cj > 0:
                    kT_sb = kvp.tile([128, Cj * 128], BF16, name="kT_sb", tag="kT")
                    nc.scalar.dma_start(
                        kT_sb[:], kT_i[:, koff[j] * 128:(koff[j] + Cj) * 128])
                    v_sb = kvp.tile([128, Cj, 129], BF16, name="v_sb", tag="v")
                    nc.gpsimd.dma_start(
                        v_sb[:],
                        v_i[:, koff[j] * 129:(koff[j] + Cj) * 129]
                        .rearrange("p (c d) -> p c d", d=129))

                    sc = scps.tile([128, Cj * 32], F32, name="sc", tag="sc")
                    for c in range(Cj):
                        nc.tensor.matmul(sc[:, c * 32:(c + 1) * 32],
                                         kT_sb[:, c * 128:(c + 1) * 128],
                                         qT_b, start=True, stop=True)
                    pp = ppp.tile([128, Cj * 32], BF16, name="pp", tag="pp")
                    for c0 in range(0, Cj, 16):
                        ce = min(c0 + 16, Cj)
                        nc.scalar.activation(pp[:, c0 * 32:ce * 32],
                                             sc[:, c0 * 32:ce * 32], AF.Exp)

                cur = curps.tile([1, 32], F32, name="cur", tag="cur")
                nc.tensor.matmul(cur[:], kcurT[:, j:j + 1], qT_b,
                                 start=True, stop=True)
                pcur = smb.tile([1, 32], BF16, name="pcur", tag="pcur")
                nc.scalar.activation(pcur[:], cur[:], AF.Exp)

                ctx = ctxps.tile([32, 129], F32, name="ctx", tag="ctx")
                for c in range(Cj):
                    nc.tensor.matmul(ctx[:], pp[:, c * 32:(c + 1) * 32],
                                     v_sb[:, c, :], start=(c == 0), stop=False)
                nc.tensor.matmul(ctx[:], pcur[:],
                                 vcur[:, j * 129:(j + 1) * 129],
                                 start=(Cj == 0), stop=True)

                rden = smb.tile([32, 1], F32, name="rden", tag="rden")
                nc.vector.reciprocal(rden[:], ctx[:, 128:129])
                ctn_sb = smb.tile([32, 128], BF16, name="ctn_sb", tag="ctn")
                nc.vector.tensor_scalar_mul(ctn_sb[:], ctx[:, :128], rden[:])
                nc.scalar.dma_start(
                    ctx_b[j:j + 1, :].rearrange("o (h d) -> h (o d)", d=HD),
                    ctn_sb[:])

            nc.gpsimd.collective_compute(
                "AllGather", ALU.bypass, replica_groups=rg,
                ins=[ctx_b.opt()], outs=[agc.opt()],
            )

        # w2 stream (Pool queue, after attention's v loads)
        w2p = top.enter_context(tc.tile_pool(name="w2p", bufs=3))
        w2_tiles = []
        for mk in range(11):
            w2t = w2p.tile([128, HID], BF16, name="w2t", tag="w2")
            nc.gpsimd.dma_start(w2t[:], w2T_i[mk * 128:(mk + 1) * 128, :])
            w2_tiles.append(w2t)

        # ---------------- Phase C: AG ctx -> Wo -> residual -> AG hidden ------
        with ExitStack() as pc:
            sC = pc.enter_context(tc.tile_pool(name="sC", bufs=1))
            wops = pc.enter_context(tc.tile_pool(name="wops", bufs=1, space="PSUM"))
            ptC = pc.enter_context(tc.tile_pool(name="ptC", bufs=2, space="PSUM"))

            hidc = sC.tile([64, 512], F32)
            nc.gpsimd.dma_start(hidc[:], hidc_i[:])
            ctx_all = sC.tile([B, HID], BF16)
            nc.scalar.dma_start(ctx_all[:], agc[:])
            ctxA = sC.tile([128, 32 * 64], BF16)
            for k in range(32):
                pt = ptC.tile([128, 64], BF16, name="ptC_t", tag="ptC_t")
                nc.tensor.transpose(pt[:], ctx_all[:, k * 128:(k + 1) * 128],
                                    id64b[:])
                nc.vector.tensor_copy(ctxA[:, k * 64:(k + 1) * 64], pt[:])

            wo_ps = wops.tile([64, 512], F32)
            for k in range(32):
                nc.tensor.matmul(wo_ps[:], ctxA[:, k * 64:(k + 1) * 64],
                                 wo_t[:, k, :], start=(k == 0), stop=(k == 31))
            hsl = sC.tile([64, 512], F32)
            nc.vector.tensor_tensor(hsl[:], wo_ps[:], hidc[:], op=ALU.add)
            nc.scalar.dma_start(hsl_b[:], hsl[:])
            nc.gpsimd.collective_compute(
                "AllGather", ALU.bypass, replica_groups=rg,
                ins=[hsl_b.opt()], outs=[agh.opt()],
            )
            nc.scalar.dma_start(hid_full[:].rearrange("b (r o) -> b r o", r=8),
                                agh.rearrange("(r b) o -> b r o", b=64))

        # ------- Phase D/E1: xT transposes + w1 + gate (interleaved) ----------
        wsel_col = small.tile([64, 1], F32, name="wsel_col")
        rs2 = small.tile([64, 1], F32, name="rs2")
        with ExitStack() as pe1:
            sD = pe1.enter_context(tc.tile_pool(name="sD", bufs=1))
            ptD = pe1.enter_context(tc.tile_pool(name="ptD", bufs=1, space="PSUM"))
            gups = pe1.enter_context(tc.tile_pool(name="gups", bufs=1, space="PSUM"))
            gps = pe1.enter_context(tc.tile_pool(name="gps", bufs=1, space="PSUM"))

            ssq2 = small.tile([64, 1], F32, name="ssq2")
            nc.scalar.activation(scratch[:], hid_full[:], AF.Square,
                                 accum_out=ssq2[:])
            nc.scalar.activation(rs2[:], ssq2[:], AF.Sqrt, bias=float(HID) * EPS)
            nc.vector.reciprocal(rs2[:], rs2[:])

            gu = gups.tile([64, TWO_MI], F32)
            g_ps = gps.tile([64, 8], F32)
            slices = [(o * 512, min(512, TWO_MI - o * 512)) for o in range(6)]
            for k in range(32):
                pt = ptD.tile([128, 64], F32, name="ptD_t", tag="ptD_t")
                nc.tensor.transpose(pt[:], hid_full[:, k * 128:(k + 1) * 128],
                                    id64f[:])
                nc.vector.tensor_copy(xT[:, k * 64:(k + 1) * 64], pt[:])
                w1t = w1_tiles[k]
                for (off, w) in slices:
                    nc.tensor.matmul(gu[:, off:off + w],
                                     xT[:, k * 64:(k + 1) * 64],
                                     w1t[:, off:off + w],
                                     start=(k == 0), stop=(k == 31))
                nc.tensor.matmul(g_ps[:], xT[:, k * 64:(k + 1) * 64],
                                 gw_t[:, k, :], start=(k == 0), stop=(k == 31))

            gu_s = sD.tile([64, TWO_MI], BF16)
            nc.vector.tensor_scalar_mul(gu_s[:], gu[:], rs2[:])
            sg = sD.tile([64, MI], BF16)
            nc.scalar.activation(sg[:], gu_s[:, :MI], AF.Silu)
            mid = sD.tile([64, MI], BF16)
            nc.vector.tensor_tensor(mid[:], sg[:], gu_s[:, MI:], op=ALU.mult)

            for mk in range(11):
                pt = ptD.tile([128, 64], BF16, name="ptE_t", tag="ptD_t")
                nc.tensor.transpose(pt[:], mid[:, mk * 128:(mk + 1) * 128],
                                    id64b[:])
                nc.vector.tensor_copy(midT[:, mk * 64:(mk + 1) * 64], pt[:])

            # gate softmax + top-2 + renormalize + per-core select (fp32)
            pg = sD.tile([64, 8], F32)
            nc.scalar.activation(pg[:], g_ps[:], AF.Exp, scale=rs2[:])
            m1c = sD.tile([64, 1], F32)
            nc.vector.reduce_max(m1c[:], pg[:], axis=mybir.AxisListType.X)
            eq1 = sD.tile([64, 8], F32)
            nc.vector.tensor_scalar(eq1[:], pg[:], m1c[:], None, op0=ALU.is_ge)
            t1 = sD.tile([64, 8], F32)
            nc.vector.tensor_tensor(t1[:], pg[:], eq1[:], op=ALU.mult)
            nc.vector.tensor_tensor(t1[:], pg[:], t1[:], op=ALU.subtract)
            m2c = sD.tile([64, 1], F32)
            nc.vector.reduce_max(m2c[:], t1[:], axis=mybir.AxisListType.X)
            keep = sD.tile([64, 8], F32)
            nc.vector.tensor_scalar(keep[:], pg[:], m2c[:], None, op0=ALU.is_ge)
            wsum = sD.tile([64, 1], F32)
            nc.vector.tensor_tensor(wsum[:], m1c[:], m2c[:], op=ALU.add)
            nc.vector.reciprocal(wsum[:], wsum[:])
            wts = sD.tile([64, 8], F32)
            nc.vector.tensor_tensor(wts[:], pg[:], keep[:], op=ALU.mult)
            nc.vector.tensor_scalar_mul(wts[:], wts[:], wsum[:])
            nc.vector.tensor_tensor(wts[:], wts[:], sel_bc[:], op=ALU.mult)
            nc.vector.reduce_sum(wsel_col[:], wts[:], axis=mybir.AxisListType.X)

        # ---------------- Phase E2: w2 + combine + AllReduce ----------------
        with ExitStack() as pe2:
            mops = pe2.enter_context(tc.tile_pool(name="mops", bufs=2, space="PSUM"))
            sF = pe2.enter_context(tc.tile_pool(name="sF", bufs=1))
            mo0 = mops.tile([64, 2048], F32, name="mo0", tag="mo")
            mo1 = mops.tile([64, 2048], F32, name="mo1", tag="mo")
            for mk in range(11):
                w2t = w2_tiles[mk]
                for oh, mo in ((0, mo0), (1, mo1)):
                    for oc in range(4):
                        off = oh * 2048 + oc * 512
                        nc.tensor.matmul(mo[:, oc * 512:(oc + 1) * 512],
                                         midT[:, mk * 64:(mk + 1) * 64],
                                         w2t[:, off:off + 512],
                                         start=(mk == 0), stop=(mk == 10))
            moe_sb = sF.tile([64, HID], BF16)
            nc.vector.tensor_scalar_mul(moe_sb[:, :2048], mo0[:], wsel_col[:])
            nc.vector.tensor_scalar_mul(moe_sb[:, 2048:], mo1[:], wsel_col[:])

            nc.scalar.dma_start(moe_b[:], moe_sb[:])
            nc.gpsimd.collective_compute(
                "AllReduce", ALU.add, replica_groups=rg,
                ins=[moe_b.opt()], outs=[ar_o.opt()],
            )
            ar_sb = sF.tile([B, HID], BF16)
            nc.scalar.dma_start(ar_sb[:], ar_o[:])
            nc.vector.tensor_tensor(scratch[:], ar_sb[:], hid_full[:], op=ALU.add)
            nc.scalar.dma_start(out_o[:], scratch[:])

    nc.compile()
    return nc


_NC_CACHE = {}


def _get_program(C):
    if C not in _NC_CACHE:
        _NC_CACHE[C] = _build_program(C)
    return _NC_CACHE[C]


def kernel(hidden_states, positions, k_cache, v_cache, seq_lens,
           norm1_w, norm2_w, Wqkv, Wo, gate_w, w1, w2):
    global LAST_RESULT
    sl = np.asarray(seq_lens, np.int64)
    perm, C = _plan(sl)
    nc = _get_program(C)
    CSUM = sum(C)
    koff = np.concatenate([[0], np.cumsum(C)]).astype(int)

    hs_all = np.asarray(hidden_states, np.float32).reshape(B, HID)
    hs = hs_all[perm]                                # permuted token order
    scale = np.float32(HD) ** -0.5
    n1 = np.asarray(norm1_w, np.float32) * 64.0
    n2 = np.asarray(norm2_w, np.float32) * 64.0

    wq_full = np.asarray(Wqkv, np.float32)
    # hidT[p, k*64+b] = hs[b, 128k+p]
    hidT = np.ascontiguousarray(
        hs.T.reshape(32, 128, 64).transpose(1, 0, 2).reshape(128, 32 * 64)
    ).astype(BF)

    kc = np.asarray(k_cache, np.float32)
    vc = np.asarray(v_cache, np.float32)
    gT = (np.asarray(gate_w, np.float32) * n2[None, :]).T  # (4096, 8)
    gTt = np.ascontiguousarray(gT.reshape(32, 128, 8).transpose(1, 0, 2)).astype(BF)
    Wo_f = np.asarray(Wo, np.float32)
    w1_f = np.asarray(w1, np.float32)
    w2_f = np.asarray(w2, np.float32)

    in_maps = []
    for c in range(NC_):
        # qkv rows for this core: q rows [c*512,(c+1)*512) (scaled) + k rows
        rows = np.concatenate([
            wq_full[c * QR_Q:(c + 1) * QR_Q] * scale,
            wq_full[NH * HD + c * QR_K: NH * HD + (c + 1) * QR_K],
        ]) * n1[None, :]                              # (528, 4096)
        wqkvT = np.ascontiguousarray(
            rows.T.reshape(32, 128, RPC).transpose(1, 0, 2)).astype(BF)

        woT = np.ascontiguousarray(
            Wo_f[c * 512:(c + 1) * 512].T.reshape(32, 128, 512)
            .transpose(1, 0, 2)).astype(BF)

        kT_buf = np.zeros((128, max(CSUM, 1) * 128), BF)
        v_buf = np.zeros((128, max(CSUM, 1) * 129), BF)
        for j in range(NB):
            Cj = C[j]
            if Cj == 0:
                continue
            b = perm[c * NB + j]
            n_real = int(sl[b]) - 1                   # positions [0, sl-1)
            span = Cj * 128
            kchunk = np.zeros((span, HD), np.float32)
            kchunk[:n_real] = kc[b, :n_real]
            kT_buf[:, koff[j] * 128: koff[j] * 128 + span] = \
                kchunk.T.astype(BF)
            vchunk = np.zeros((Cj, 128, 129), np.float32)
            vflat = vchunk.reshape(span, 129)
            vflat[:n_real, :HD] = vc[b, :n_real]
            vflat[:n_real, HD] = 1.0
            v_buf[:, koff[j] * 129: (koff[j] + Cj) * 129] = \
                vchunk.transpose(1, 0, 2).reshape(128, Cj * 129).astype(BF)

        sel = np.zeros((1, 8), np.float32)
        sel[0, c] = 1.0
        in_maps.append({
            "hid": hs,
            "hidT": hidT,
            "hidcols": np.ascontiguousarray(hs[:, c * 512:(c + 1) * 512]),
            "wqkvT": wqkvT,
            "woT": woT,
            "gateT": gTt,
            "w1T": np.ascontiguousarray((w1_f[c] * n2[None, :]).T).astype(BF),
            "w2T": np.ascontiguousarray(w2_f[c].T).astype(BF),
            "kT": kT_buf,
            "v": v_buf,
            "sel": sel,
        })

    LAST_RESULT = run_bass_kernel_spmd(nc, in_maps, core_ids=list(range(NC_)))
    res = LAST_RESULT.results[0]["out"]               # (64, 4096), permuted
    out = np.empty((B, HID), np.float32)
    out[perm] = res
    return out.reshape(B, 1, HID).astype(np.float32)


# revision 24
# speedup vs baseline: 1.9258x; 1.8016x over previous
"""DeepSeek-V2 decode layer on 8 TRN2 NeuronCores (Bass/Tile SPMD kernel).

v2 design (bf16 + seq-truncated attention + balanced batch placement):
  - All matmul operands bf16 (fp32 PSUM accumulate); residual stream, softmax
    denominators, and gate top-2 stay fp32.  bf16-everything sim rel-err vs
    the fp32 reference is ~2.4e-3 (budget 2e-2).
  - QKV proj row-parallel (512 q rows + 16 current-k rows per core), AllToAll
    (bf16) redistributes q^T/kcur^T to batch-sharded layout.
  - Attention data-parallel: 8 sequences per core, chosen by LPT bin-packing
    on ceil((seq_len-1)/128) so all cores get the same per-slot chunk budget
    C_j (required for SPMD) with minimal padding.  Host zero-pads K^T columns
    and V rows outside [0, seq_len-1); V carries a ones-column so the softmax
    denominator falls out of the ctx matmul for free.  The current token's
    k==v vector is applied via tiny rank-1 matmuls (uniform across cores).
  - ctx matmul uses p-chunks as stationary so ctx lands directly as
    (head, dim) -- no per-batch transpose, no DRAM broadcast round-trip.
  - Wo output-column-parallel (512 cols/core) after bf16 AllGather of ctx;
    post-attn hidden AllGather in fp32 (residual precision).
  - MoE expert-parallel (1 expert/core), norm2 weights folded into w1/gate
    host-side and the 1/rms per-token scale applied on gu -- so w1 matmuls
    start right after the hidden AllGather without waiting on the rsqrt.
    Expert outputs combined via bf16 AllReduce.
  - Bulk weight DMA spread across the SP (w1) and Pool (wo, gate, w2) HWDGE
    queues; latency-critical per-phase traffic rides ACT and DVE queues.
"""

import os
import sys

import numpy as np

for _p in ("/opt/trn_rl_repo", "/root/.axon_site/_ro/trn_rl_repo", "/root/.axon_site"):
    if _p not in sys.path and os.path.isdir(_p):
        sys.path.append(_p)


def _ensure_ntff_hook():
    """This image's antenv lacks axon_hooks; shim it so BASS_TRACE works."""
    import types

    try:
        import antenv.axon_hooks  # noqa: F401
        return
    except ImportError:
        pass
    import antenv

    mod = types.ModuleType("antenv.axon_hooks")
    _state = {"h": None}
    mod.set_axon_ntff_profile_hook = lambda h: _state.__setitem__("h", h)
    mod.get_axon_ntff_profile_hook = lambda: _state["h"]
    sys.modules["antenv.axon_hooks"] = mod
    antenv.axon_hooks = mod
    try:
        sys.path.insert(0, "/root/.axon_site/trn_agent_boot")
        import trn_boot

        so_path = "/opt/axon/libaxon_pjrt.so"
        if os.path.exists(so_path):
            mod.set_axon_ntff_profile_hook(
                trn_boot._ntff_profile_via_ctypes(so_path))
    except Exception as e:  # tracing degrades; compile+run still work
        print(f"ntff hook install failed: {e}")


_ensure_ntff_hook()

import ml_dtypes
import concourse.bacc as bacc
import concourse.bass as bass
import concourse.mybir as mybir
import concourse.tile as tile
from concourse.bass_utils import run_bass_kernel_spmd
from concourse.masks import make_identity
from contextlib import ExitStack

F32 = mybir.dt.float32
BF16 = mybir.dt.bfloat16
AF = mybir.ActivationFunctionType
ALU = mybir.AluOpType
BF = ml_dtypes.bfloat16

B, HID, S, NH, HD = 64, 4096, 4096, 32, 128
MI, TWO_MI = 1408, 2816
NC_ = 8
NB = B // NC_                 # 8 local batches (slots) per core
QR_Q, QR_K = 512, 16          # per-core q rows / current-k rows of Wqkv
RPC = QR_Q + QR_K             # 528
EPS = 1e-6

LAST_RESULT = None


def _plan(seq_lens):
    """Slot budgets + batch->core placement balanced on KV chunk count.

    Real attended positions per batch are [0, sl-1) (the current token is
    handled separately), so cb = ceil((sl-1)/128).  Sort desc, group ranks
    [8j, 8j+8) into slot j with budget C[j] = group max; core c takes the
    c-th member of each group.  perm[c*8+j] = global batch index.
    """
    sl = np.asarray(seq_lens, np.int64)
    cb = np.maximum(sl - 1, 0)
    cb = -(-cb // 128)
    order = np.argsort(-cb, kind="stable")
    C = [int(cb[order[j * 8]]) for j in range(NB)]
    perm = np.empty(B, np.int64)
    for j in range(NB):
        for c in range(NC_):
            perm[c * NB + j] = order[j * 8 + c]
    return perm, tuple(C)


def _build_program(C):
    nc = bacc.Bacc(None, target_bir_lowering=False, num_devices=NC_)

    CSUM = sum(C)
    koff = np.concatenate([[0], np.cumsum(C)]).astype(int)  # chunk offsets

    hid_i = nc.dram_tensor("hid", [B, HID], F32, kind="ExternalInput")
    hidT_i = nc.dram_tensor("hidT", [128, 32 * 64], BF16, kind="ExternalInput")
    hidc_i = nc.dram_tensor("hidcols", [B, 512], F32, kind="ExternalInput")
    wqkvT_i = nc.dram_tensor("wqkvT", [128, 32, RPC], BF16, kind="ExternalInput")
    woT_i = nc.dram_tensor("woT", [128, 32, 512], BF16, kind="ExternalInput")
    gateT_i = nc.dram_tensor("gateT", [128, 32, 8], BF16, kind="ExternalInput")
    w1T_i = nc.dram_tensor("w1T", [HID, TWO_MI], BF16, kind="ExternalInput")
    w2T_i = nc.dram_tensor("w2T", [MI, HID], BF16, kind="ExternalInput")
    kT_i = nc.dram_tensor("kT", [128, max(CSUM, 1) * 128], BF16,
                          kind="ExternalInput")
    v_i = nc.dram_tensor("v", [128, max(CSUM, 1) * 129], BF16,
                         kind="ExternalInput")
    sel_i = nc.dram_tensor("sel", [1, 8], F32, kind="ExternalInput")
    out_o = nc.dram_tensor("out", [B, HID], F32, kind="ExternalOutput")

    rg = [list(range(NC_))]

    with tile.TileContext(nc) as tc, ExitStack() as top:
        # A2A payload is token-major: block for dest core s = qkv_sb rows
        # [s*8,(s+1)*8) -- a single contiguous SBUF->DRAM copy.  The q
        # transpose happens consumer-side (4 PE transposes); the resulting
        # head-block order h' = jt*8 + r (actual head r*4+jt) is absorbed
        # into the host-side layout of Wo.
        dramp = top.enter_context(tc.tile_pool(name="dram", bufs=1, space="DRAM"))
        a2a_in = dramp.tile([NC_, NB, RPC], BF16)
        a2a_out = dramp.tile([NC_, NB, RPC], BF16)
        ctx_b = dramp.tile([NB, HID], BF16)
        agc = dramp.tile([B, HID], BF16, addr_space="Shared")
        hsl_b = dramp.tile([B, 512], F32)
        agh = dramp.tile([B * 8, 512], F32, addr_space="Shared")
        moe_b = dramp.tile([B, HID], BF16)
        ar_o = dramp.tile([B, HID], BF16, addr_space="Shared")

        const = top.enter_context(tc.tile_pool(name="const", bufs=1))
        id64b = const.tile([64, 64], BF16)
        make_identity(nc, id64b)
        id64f = const.tile([64, 64], F32)
        make_identity(nc, id64f)
        zero_col = const.tile([128, 1], F32)
        nc.gpsimd.memset(zero_col[:], 0.0)
        eps_col = const.tile([128, 1], F32)
        nc.gpsimd.memset(eps_col[:], float(HID) * EPS)
        nc.const_aps.aps[(F32, 0.0)] = zero_col[:]
        nc.const_aps.aps[(F32, float(HID) * EPS)] = eps_col[:]
        sel_bc = const.tile([64, 8], F32)
        nc.gpsimd.dma_start(sel_bc[:], sel_i.ap().to_broadcast((64, 8)))

        # long-lived activations
        acts = top.enter_context(tc.tile_pool(name="acts", bufs=1))
        scratch = acts.tile([B, HID], F32)       # square scratch / final out
        hid_full = acts.tile([B, HID], F32)      # post-attn residual hidden
        xT = acts.tile([128, 32 * 64], BF16)     # post-attn hidden transposed
        midT = acts.tile([128, 11 * 64], BF16)
        small = top.enter_context(tc.tile_pool(name="small", bufs=1))

        # ---------------- Phase A: norm1 -> qkv -> A2A ----------------
        with ExitStack() as pa:
            sA = pa.enter_context(tc.tile_pool(name="sA", bufs=1))
            qkvps = pa.enter_context(tc.tile_pool(name="qkvps", bufs=1, space="PSUM"))

            hT = sA.tile([128, 32 * 64], BF16)
            nc.scalar.dma_start(hT[:], hidT_i[:])
            h_sb = sA.tile([B, HID], F32)
            nc.scalar.dma_start(h_sb[:], hid_i[:])
            wq = sA.tile([128, 32, RPC], BF16)
            nc.scalar.dma_start(wq[:, :16], wqkvT_i[:, :16])
            nc.sync.dma_start(wq[:, 16:], wqkvT_i[:, 16:])

            ssq = small.tile([64, 1], F32, name="ssq")
            nc.scalar.activation(scratch[:], h_sb[:], AF.Square, accum_out=ssq[:])
            rs_col = small.tile([64, 1], F32, name="rs_col")
            nc.scalar.activation(rs_col[:], ssq[:], AF.Sqrt, bias=float(HID) * EPS)
            nc.vector.reciprocal(rs_col[:], rs_col[:])

            q1 = qkvps.tile([64, 512], F32, name="q1")
            q2 = qkvps.tile([64, 16], F32, name="q2")
            for k in range(32):
                nc.tensor.matmul(q1[:], hT[:, k * 64:(k + 1) * 64],
                                 wq[:, k, :512], start=(k == 0), stop=(k == 31))
                nc.tensor.matmul(q2[:], hT[:, k * 64:(k + 1) * 64],
                                 wq[:, k, 512:RPC], start=(k == 0), stop=(k == 31))
            qkv_sb = sA.tile([64, RPC], BF16)
            nc.vector.tensor_scalar_mul(qkv_sb[:, :512], q1[:], rs_col[:])
            nc.vector.tensor_scalar_mul(qkv_sb[:, 512:RPC], q2[:], rs_col[:])

            nc.scalar.dma_start(
                a2a_in[:].rearrange("s j x -> (s j) x"), qkv_sb[:])

            nc.gpsimd.collective_compute(
                "AllToAll", ALU.bypass, replica_groups=rg,
                ins=[a2a_in.opt()], outs=[a2a_out.opt()],
            )

        # ------- bulk weight streams (SP + Pool queues, consumption order) ----
        w1p = top.enter_context(tc.tile_pool(name="w1p", bufs=5))
        w1_tiles = []
        for k in range(32):
            w1t = w1p.tile([128, TWO_MI], BF16, name="w1t", tag="w1")
            nc.sync.dma_start(w1t[:], w1T_i[k * 128:(k + 1) * 128, :])
            w1_tiles.append(w1t)
        wop = top.enter_context(tc.tile_pool(name="wop", bufs=1))
        wo_t = wop.tile([128, 32, 512], BF16)
        nc.gpsimd.dma_start(wo_t[:], woT_i[:])
        gwp = top.enter_context(tc.tile_pool(name="gwp", bufs=1))
        gw_t = gwp.tile([128, 32, 8], BF16)
        nc.gpsimd.dma_start(gw_t[:], gateT_i[:])
        # ---------------- Phase B: attention (8 slots) ----------------
        with ExitStack() as pb:
            sB = pb.enter_context(tc.tile_pool(name="sB", bufs=1))
            kvp = pb.enter_context(tc.tile_pool(name="kvp", bufs=2))
            ppp = pb.enter_context(tc.tile_pool(name="ppp", bufs=2))
            smb = pb.enter_context(tc.tile_pool(name="smb", bufs=2))
            scps = pb.enter_context(tc.tile_pool(name="scps", bufs=2, space="PSUM"))
            curps = pb.enter_context(tc.tile_pool(name="curps", bufs=2, space="PSUM"))
            ctxps = pb.enter_context(tc.tile_pool(name="ctxps", bufs=2, space="PSUM"))

            # qcols[(r*8+j), x] = source core r's qkv row x for my token j
            qcols = sB.tile([64, RPC], BF16)
            nc.scalar.dma_start(
                qcols[:], a2a_out[:].rearrange("r j x -> (r j) x"))
            # tpA[p, jt, r*8+j]: q for head-block h' = jt*8+r (head r*4+jt)
            tpA = sB.tile([128, 4, 64], BF16)
            for jt in range(4):
                pt = curps.tile([128, 64], BF16, name="ptB_q", tag="cur")
                nc.tensor.transpose(pt[:], qcols[:, jt * 128:(jt + 1) * 128],
                                    id64b[:])
                nc.vector.tensor_copy(tpA[:, jt, :], pt[:])
            kcurT = sB.tile([128, NB], BF16)
            vcur = sB.tile([1, NB * 129], BF16)
            for r in range(NC_):
                kblk = a2a_out[r, :, 512:RPC]           # (8 tokens, 16 d)
                nc.scalar.dma_start(
                    kcurT[r * QR_K:(r + 1) * QR_K, :],
                    kblk.rearrange("j t -> t j"))
                nc.scalar.dma_start(
                    vcur[:].rearrange("o (j x) -> o j x", x=129)
                    [:, :, r * QR_K:(r + 1) * QR_K],
                    kblk)
            nc.gpsimd.memset(
                vcur[:].rearrange("o (j x) -> o j x", x=129)[:, :, 128:129], 1.0)

            for j in range(NB):
                Cj = C[j]
                qT_b = (tpA[:].rearrange("p q (r i) -> p q r i", i=8)
                        [:, :, :, j])
                if Cj > 0:
                    kT_sb = kvp.tile([128, Cj * 128], BF16, name="kT_sb", tag="kT")
                    nc.scalar.dma_start(
                        kT_sb[:], kT_i[:, koff[j] * 128:(koff[j] + Cj) * 128])
                    v_sb = kvp.tile([128, Cj, 129], BF16, name="v_sb", tag="v")
                    nc.gpsimd.dma_start(
                        v_sb[:],
                        v_i[:, koff[j] * 129:(koff[j] + Cj) * 129]
                        .rearrange("p (c d) -> p c d", d=129))

                    sc = scps.tile([128, Cj * 32], F32, name="sc", tag="sc")
                    for c in range(Cj):
                        nc.tensor.matmul(sc[:, c * 32:(c + 1) * 32],
                                         kT_sb[:, c * 128:(c + 1) * 128],
                                         qT_b, start=True, stop=True)
                    pp = ppp.tile([128, Cj * 32], BF16, name="pp", tag="pp")
                    for c0 in range(0, Cj, 16):
                        ce = min(c0 + 16, Cj)
                        nc.scalar.activation(pp[:, c0 * 32:ce * 32],
                                             sc[:, c0 * 32:ce * 32], AF.Exp)

                cur = curps.tile([1, 32], F32, name="cur", tag="cur")
                nc.tensor.matmul(cur[:], kcurT[:, j:j + 1], qT_b,
                                 start=True, stop=True)
                pcur = smb.tile([1, 32], BF16, name="pcur", tag="pcur")
                nc.scalar.activation(pcur[:], cur[:], AF.Exp)

                ctx = ctxps.tile([32, 129], F32, name="ctx", tag="ctx")
                for c in range(Cj):
                    nc.tensor.matmul(ctx[:], pp[:, c * 32:(c + 1) * 32],
                                     v_sb[:, c, :], start=(c == 0), stop=False)
                nc.tensor.matmul(ctx[:], pcur[:],
                                 vcur[:, j * 129:(j + 1) * 129],
                                 start=(Cj == 0), stop=True)

                rden = smb.tile([32, 1], F32, name="rden", tag="rden")
                nc.vector.reciprocal(rden[:], ctx[:, 128:129])
                ctn_sb = smb.tile([32, 128], BF16, name="ctn_sb", tag="ctn")
                nc.vector.tensor_scalar_mul(ctn_sb[:], ctx[:, :128], rden[:])
                nc.scalar.dma_start(
                    ctx_b[j:j + 1, :].rearrange("o (h d) -> h (o d)", d=HD),
                    ctn_sb[:])

            nc.gpsimd.collective_compute(
                "AllGather", ALU.bypass, replica_groups=rg,
                ins=[ctx_b.opt()], outs=[agc.opt()],
            )

        # w2 stream (Pool queue, after attention's v loads)
        w2p = top.enter_context(tc.tile_pool(name="w2p", bufs=3))
        w2_tiles = []
        for mk in range(11):
            w2t = w2p.tile([128, HID], BF16, name="w2t", tag="w2")
            nc.gpsimd.dma_start(w2t[:], w2T_i[mk * 128:(mk + 1) * 128, :])
            w2_tiles.append(w2t)

        # ---------------- Phase C: AG ctx -> Wo -> residual -> AG hidden ------
        with ExitStack() as pc:
            sC = pc.enter_context(tc.tile_pool(name="sC", bufs=1))
            wops = pc.enter_context(tc.tile_pool(name="wops", bufs=1, space="PSUM"))
            ptC = pc.enter_context(tc.tile_pool(name="ptC", bufs=2, space="PSUM"))

            hidc = sC.tile([64, 512], F32)
            nc.gpsimd.dma_start(hidc[:], hidc_i[:])
            ctx_all = sC.tile([B, HID], BF16)
            nc.scalar.dma_start(ctx_all[:], agc[:])
            ctxA = sC.tile([128, 32 * 64], BF16)
            for k in range(32):
                pt = ptC.tile([128, 64], BF16, name="ptC_t", tag="ptC_t")
                nc.tensor.transpose(pt[:], ctx_all[:, k * 128:(k + 1) * 128],
                                    id64b[:])
                nc.vector.tensor_copy(ctxA[:, k * 64:(k + 1) * 64], pt[:])

            wo_ps = wops.tile([64, 512], F32)
            for k in range(32):
                nc.tensor.matmul(wo_ps[:], ctxA[:, k * 64:(k + 1) * 64],
                                 wo_t[:, k, :], start=(k == 0), stop=(k == 31))
            hsl = sC.tile([64, 512], F32)
            nc.vector.tensor_tensor(hsl[:], wo_ps[:], hidc[:], op=ALU.add)
            nc.scalar.dma_start(hsl_b[:], hsl[:])
            nc.gpsimd.collective_compute(
                "AllGather", ALU.bypass, replica_groups=rg,
                ins=[hsl_b.opt()], outs=[agh.opt()],
            )
            nc.scalar.dma_start(hid_full[:].rearrange("b (r o) -> b r o", r=8),
                                agh.rearrange("(r b) o -> b r o", b=64))

        # ------- Phase D/E1: xT transposes + w1 + gate (interleaved) ----------
        wsel_col = small.tile([64, 1], F32, name="wsel_col")
        rs2 = small.tile([64, 1], F32, name="rs2")
        with ExitStack() as pe1:
            sD = pe1.enter_context(tc.tile_pool(name="sD", bufs=1))
            ptD = pe1.enter_context(tc.tile_pool(name="ptD", bufs=1, space="PSUM"))
            gups = pe1.enter_context(tc.tile_pool(name="gups", bufs=1, space="PSUM"))
            gps = pe1.enter_context(tc.tile_pool(name="gps", bufs=1, space="PSUM"))

            ssq2 = small.tile([64, 1], F32, name="ssq2")
            nc.scalar.activation(scratch[:], hid_full[:], AF.Square,
                                 accum_out=ssq2[:])
            nc.scalar.activation(rs2[:], ssq2[:], AF.Sqrt, bias=float(HID) * EPS)
            nc.vector.reciprocal(rs2[:], rs2[:])

            gu = gups.tile([64, TWO_MI], F32)
            g_ps = gps.tile([64, 8], F32)
            slices = [(o * 512, min(512, TWO_MI - o * 512)) for o in range(6)]
            for k in range(32):
                pt = ptD.tile([128, 64], F32, name="ptD_t", tag="ptD_t")
                nc.tensor.transpose(pt[:], hid_full[:, k * 128:(k + 1) * 128],
                                    id64f[:])
                nc.vector.tensor_copy(xT[:, k * 64:(k + 1) * 64], pt[:])
                w1t = w1_tiles[k]
                for (off, w) in slices:
                    nc.tensor.matmul(gu[:, off:off + w],
                                     xT[:, k * 64:(k + 1) * 64],
                                     w1t[:, off:off + w],
                                     start=(k == 0), stop=(k == 31))
                nc.tensor.matmul(g_ps[:], xT[:, k * 64:(k + 1) * 64],
                                 gw_t[:, k, :], start=(k == 0), stop=(k == 31))

            gu_s = sD.tile([64, TWO_MI], BF16)
            nc.vector.tensor_scalar_mul(gu_s[:], gu[:], rs2[:])
            sg = sD.tile([64, MI], BF16)
            nc.scalar.activation(sg[:], gu_s[:, :MI], AF.Silu)
            mid = sD.tile([64, MI], BF16)
            nc.vector.tensor_tensor(mid[:], sg[:], gu_s[:, MI:], op=ALU.mult)

            for mk in range(11):
                pt = ptD.tile([128, 64], BF16, name="ptE_t", tag="ptD_t")
                nc.tensor.transpose(pt[:], mid[:, mk * 128:(mk + 1) * 128],
                                    id64b[:])
                nc.vector.tensor_copy(midT[:, mk * 64:(mk + 1) * 64], pt[:])

            # gate softmax + top-2 + renormalize + per-core select (fp32)
            pg = sD.tile([64, 8], F32)
            nc.scalar.activation(pg[:], g_ps[:], AF.Exp, scale=rs2[:])
            m1c = sD.tile([64, 1], F32)
            nc.vector.reduce_max(m1c[:], pg[:], axis=mybir.AxisListType.X)
            eq1 = sD.tile([64, 8], F32)
            nc.vector.tensor_scalar(eq1[:], pg[:], m1c[:], None, op0=ALU.is_ge)
            t1 = sD.tile([64, 8], F32)
            nc.vector.tensor_tensor(t1[:], pg[:], eq1[:], op=ALU.mult)
            nc.vector.tensor_tensor(t1[:], pg[:], t1[:], op=ALU.subtract)
            m2c = sD.tile([64, 1], F32)
            nc.vector.reduce_max(m2c[:], t1[:], axis=mybir.AxisListType.X)
            keep = sD.tile([64, 8], F32)
            nc.vector.tensor_scalar(keep[:], pg[:], m2c[:], None, op0=ALU.is_ge)
            wsum = sD.tile([64, 1], F32)
            nc.vector.tensor_tensor(wsum[:], m1c[:], m2c[:], op=ALU.add)
            nc.vector.reciprocal(wsum[:], wsum[:])
            wts = sD.tile([64, 8], F32)
            nc.vector.tensor_tensor(wts[:], pg[:], keep[:], op=ALU.mult)
            nc.vector.tensor_scalar_mul(wts[:], wts[:], wsum[:])
            nc.vector.tensor_tensor(wts[:], wts[:], sel_bc[:], op=ALU.mult)
            nc.vector.reduce_sum(wsel_col[:], wts[:], axis=mybir.AxisListType.X)

        # ---------------- Phase E2: w2 + combine + AllReduce ----------------
        with ExitStack() as pe2:
            mops = pe2.enter_context(tc.tile_pool(name="mops", bufs=2, space="PSUM"))
            sF = pe2.enter_context(tc.tile_pool(name="sF", bufs=1))
            mo0 = mops.tile([64, 2048], F32, name="mo0", tag="mo")
            mo1 = mops.tile([64, 2048], F32, name="mo1", tag="mo")
            for mk in range(11):
                w2t = w2_tiles[mk]
                for oh, mo in ((0, mo0), (1, mo1)):
                    for oc in range(4):
                        off = oh * 2048 + oc * 512
                        nc.tensor.matmul(mo[:, oc * 512:(oc + 1) * 512],
                                         midT[:, mk * 64:(mk + 1) * 64],
                                         w2t[:, off:off + 512],
                                         start=(mk == 0), stop=(mk == 10))
            moe_sb = sF.tile([64, HID], BF16)
            nc.vector.tensor_scalar_mul(moe_sb[:, :2048], mo0[:], wsel_col[:])
            nc.vector.tensor_scalar_mul(moe_sb[:, 2048:], mo1[:], wsel_col[:])

            nc.scalar.dma_start(moe_b[:], moe_sb[:])
            nc.gpsimd.collective_compute(
                "AllReduce", ALU.add, replica_groups=rg,
                ins=[moe_b.opt()], outs=[ar_o.opt()],
            )
            ar_sb = sF.tile([B, HID], BF16)
            nc.scalar.dma_start(ar_sb[:], ar_o[:])
            nc.vector.tensor_tensor(scratch[:], ar_sb[:], hid_full[:], op=ALU.add)
            nc.scalar.dma_start(out_o[:], scratch[:])

    nc.compile()
    return nc


_NC_CACHE = {}


def _get_program(C):
    if C not in _NC_CACHE:
        _NC_CACHE[C] = _build_program(C)
    return _NC_CACHE[C]


def kernel(hidden_states, positions, k_cache, v_cache, seq_lens,
           norm1_w, norm2_w, Wqkv, Wo, gate_w, w1, w2):
    global LAST_RESULT
    sl = np.asarray(seq_lens, np.int64)
    perm, C = _plan(sl)
    nc = _get_program(C)
    CSUM = sum(C)
    koff = np.concatenate([[0], np.cumsum(C)]).astype(int)

    hs_all = np.asarray(hidden_states, np.float32).reshape(B, HID)
    hs = hs_all[perm]                                # permuted token order
    scale = np.float32(HD) ** -0.5
    n1 = np.asarray(norm1_w, np.float32) * 64.0
    n2 = np.asarray(norm2_w, np.float32) * 64.0

    wq_full = np.asarray(Wqkv, np.float32)
    # hidT[p, k*64+b] = hs[b, 128k+p]
    hidT = np.ascontiguousarray(
        hs.T.reshape(32, 128, 64).transpose(1, 0, 2).reshape(128, 32 * 64)
    ).astype(BF)

    kc = np.asarray(k_cache, np.float32)
    vc = np.asarray(v_cache, np.float32)
    gT = (np.asarray(gate_w, np.float32) * n2[None, :]).T  # (4096, 8)
    gTt = np.ascontiguousarray(gT.reshape(32, 128, 8).transpose(1, 0, 2)).astype(BF)
    Wo_f = np.asarray(Wo, np.float32)
    w1_f = np.asarray(w1, np.float32)
    w2_f = np.asarray(w2, np.float32)

    in_maps = []
    for c in range(NC_):
        # qkv rows for this core: q rows [c*512,(c+1)*512) (scaled) + k rows
        rows = np.concatenate([
            wq_full[c * QR_Q:(c + 1) * QR_Q] * scale,
            wq_full[NH * HD + c * QR_K: NH * HD + (c + 1) * QR_K],
        ]) * n1[None, :]                              # (528, 4096)
        wqkvT = np.ascontiguousarray(
            rows.T.reshape(32, 128, RPC).transpose(1, 0, 2)).astype(BF)

        # on-chip ctx head-block order h' -> actual head (h'%8)*4 + h'//8
        perm_h = np.array([(k % 8) * 4 + k // 8 for k in range(32)])
        woT = np.ascontiguousarray(
            Wo_f[c * 512:(c + 1) * 512].T.reshape(32, 128, 512)[perm_h]
            .transpose(1, 0, 2)).astype(BF)

        kT_buf = np.zeros((128, max(CSUM, 1) * 128), BF)
        v_buf = np.zeros((128, max(CSUM, 1) * 129), BF)
        for j in range(NB):
            Cj = C[j]
            if Cj == 0:
                continue
            b = perm[c * NB + j]
            n_real = int(sl[b]) - 1                   # positions [0, sl-1)
            span = Cj * 128
            kchunk = np.zeros((span, HD), np.float32)
            kchunk[:n_real] = kc[b, :n_real]
            kT_buf[:, koff[j] * 128: koff[j] * 128 + span] = \
                kchunk.T.astype(BF)
            vchunk = np.zeros((Cj, 128, 129), np.float32)
            vflat = vchunk.reshape(span, 129)
            vflat[:n_real, :HD] = vc[b, :n_real]
            vflat[:n_real, HD] = 1.0
            v_buf[:, koff[j] * 129: (koff[j] + Cj) * 129] = \
                vchunk.transpose(1, 0, 2).reshape(128, Cj * 129).astype(BF)

        sel = np.zeros((1, 8), np.float32)
        sel[0, c] = 1.0
        in_maps.append({
            "hid": hs,
            "hidT": hidT,
            "hidcols": np.ascontiguousarray(hs[:, c * 512:(c + 1) * 512]),
            "wqkvT": wqkvT,
            "woT": woT,
            "gateT": gTt,
            "w1T": np.ascontiguousarray((w1_f[c] * n2[None, :]).T).astype(BF),
            "w2T": np.ascontiguousarray(w2_f[c].T).astype(BF),
            "kT": kT_buf,
            "v": v_buf,
            "sel": sel,
        })

    LAST_RESULT = run_bass_kernel_spmd(nc, in_maps, core_ids=list(range(NC_)))
    res = LAST_RESULT.results[0]["out"]               # (64, 4096), permuted
    out = np.empty((B, HID), np.float32)
    out[perm] = res
    return out.reshape(B, 1, HID).astype(np.float32)


# revision 46
# speedup vs baseline: 2.0420x; 1.0603x over previous
"""DeepSeek-V2 decode layer on 8 TRN2 NeuronCores (Bass/Tile SPMD kernel).

v2 design (bf16 + seq-truncated attention + balanced batch placement):
  - All matmul operands bf16 (fp32 PSUM accumulate); residual stream, softmax
    denominators, and gate top-2 stay fp32.  bf16-everything sim rel-err vs
    the fp32 reference is ~2.4e-3 (budget 2e-2).
  - QKV proj row-parallel (512 q rows + 16 current-k rows per core), AllToAll
    (bf16) redistributes q^T/kcur^T to batch-sharded layout.
  - Attention data-parallel: 8 sequences per core, chosen by LPT bin-packing
    on ceil((seq_len-1)/128) so all cores get the same per-slot chunk budget
    C_j (required for SPMD) with minimal padding.  Host zero-pads K^T columns
    and V rows outside [0, seq_len-1); V carries a ones-column so the softmax
    denominator falls out of the ctx matmul for free.  The current token's
    k==v vector is applied via tiny rank-1 matmuls (uniform across cores).
  - ctx matmul uses p-chunks as stationary so ctx lands directly as
    (head, dim) -- no per-batch transpose, no DRAM broadcast round-trip.
  - Wo output-column-parallel (512 cols/core) after bf16 AllGather of ctx;
    post-attn hidden AllGather in fp32 (residual precision).
  - MoE expert-parallel (1 expert/core), norm2 weights folded into w1/gate
    host-side and the 1/rms per-token scale applied on gu -- so w1 matmuls
    start right after the hidden AllGather without waiting on the rsqrt.
    Expert outputs combined via bf16 AllReduce.
  - Bulk weight DMA spread across the SP (w1) and Pool (wo, gate, w2) HWDGE
    queues; latency-critical per-phase traffic rides ACT and DVE queues.
"""

import os
import sys

import numpy as np

for _p in ("/opt/trn_rl_repo", "/root/.axon_site/_ro/trn_rl_repo", "/root/.axon_site"):
    if _p not in sys.path and os.path.isdir(_p):
        sys.path.append(_p)


def _ensure_ntff_hook():
    """This image's antenv lacks axon_hooks; shim it so BASS_TRACE works."""
    import types

    try:
        import antenv.axon_hooks  # noqa: F401
        return
    except ImportError:
        pass
    import antenv

    mod = types.ModuleType("antenv.axon_hooks")
    _state = {"h": None}
    mod.set_axon_ntff_profile_hook = lambda h: _state.__setitem__("h", h)
    mod.get_axon_ntff_profile_hook = lambda: _state["h"]
    sys.modules["antenv.axon_hooks"] = mod
    antenv.axon_hooks = mod
    try:
        sys.path.insert(0, "/root/.axon_site/trn_agent_boot")
        import trn_boot

        so_path = "/opt/axon/libaxon_pjrt.so"
        if os.path.exists(so_path):
            mod.set_axon_ntff_profile_hook(
                trn_boot._ntff_profile_via_ctypes(so_path))
    except Exception as e:  # tracing degrades; compile+run still work
        print(f"ntff hook install failed: {e}")


_ensure_ntff_hook()

import ml_dtypes
import concourse.bacc as bacc
import concourse.bass as bass
import concourse.mybir as mybir
import concourse.tile as tile
from concourse.bass_utils import run_bass_kernel_spmd
from concourse.masks import make_identity
from contextlib import ExitStack

F32 = mybir.dt.float32
BF16 = mybir.dt.bfloat16
AF = mybir.ActivationFunctionType
ALU = mybir.AluOpType
BF = ml_dtypes.bfloat16

B, HID, S, NH, HD = 64, 4096, 4096, 32, 128
MI, TWO_MI = 1408, 2816
NC_ = 8
NB = B // NC_                 # 8 local batches (slots) per core
QR_Q, QR_K = 512, 16          # per-core q rows / current-k rows of Wqkv
RPC = QR_Q + QR_K             # 528
EPS = 1e-6

LAST_RESULT = None


def _plan(seq_lens):
    """Slot budgets + batch->core placement balanced on KV chunk count.

    Real attended positions per batch are [0, sl-1) (the current token is
    handled separately), so cb = ceil((sl-1)/128).  Sort desc, group ranks
    [8j, 8j+8) into slot j with budget C[j] = group max; core c takes the
    c-th member of each group.  perm[c*8+j] = global batch index.
    """
    sl = np.asarray(seq_lens, np.int64)
    cb = np.maximum(sl - 1, 0)
    cb = -(-cb // 128)
    order = np.argsort(-cb, kind="stable")
    C = [int(cb[order[j * 8]]) for j in range(NB)]
    perm = np.empty(B, np.int64)
    for j in range(NB):
        for c in range(NC_):
            perm[c * NB + j] = order[j * 8 + c]
    return perm, tuple(C)


def _build_program(C):
    nc = bacc.Bacc(None, target_bir_lowering=False, num_devices=NC_)

    CSUM = sum(C)
    koff = np.concatenate([[0], np.cumsum(C)]).astype(int)  # chunk offsets

    hid_i = nc.dram_tensor("hid", [B, HID], F32, kind="ExternalInput")
    hidT_i = nc.dram_tensor("hidT", [128, 32 * 64], BF16, kind="ExternalInput")
    hidc_i = nc.dram_tensor("hidcols", [B, 512], F32, kind="ExternalInput")
    wqkvT_i = nc.dram_tensor("wqkvT", [128, 32, RPC], BF16, kind="ExternalInput")
    woT_i = nc.dram_tensor("woT", [128, 32, 512], BF16, kind="ExternalInput")
    gateT_i = nc.dram_tensor("gateT", [128, 32, 8], BF16, kind="ExternalInput")
    w1T_i = nc.dram_tensor("w1T", [HID, TWO_MI], BF16, kind="ExternalInput")
    w2aT_i = nc.dram_tensor("w2aT", [MI, 2048], BF16, kind="ExternalInput")
    w2bT_i = nc.dram_tensor("w2bT", [MI, 2048], BF16, kind="ExternalInput")
    kT_i = nc.dram_tensor("kT", [128, max(CSUM, 1) * 128], BF16,
                          kind="ExternalInput")
    v_i = nc.dram_tensor("v", [128, max(CSUM, 1) * 129], BF16,
                         kind="ExternalInput")
    sel_i = nc.dram_tensor("sel", [1, 8], F32, kind="ExternalInput")
    out_o = nc.dram_tensor("out", [B, HID], F32, kind="ExternalOutput")

    rg = [list(range(NC_))]

    with tile.TileContext(nc) as tc, ExitStack() as top:
        # A2A payload is token-major: block for dest core s = qkv_sb rows
        # [s*8,(s+1)*8) -- a single contiguous SBUF->DRAM copy.  The q
        # transpose happens consumer-side (4 PE transposes); the resulting
        # head-block order h' = jt*8 + r (actual head r*4+jt) is absorbed
        # into the host-side layout of Wo.
        dramp = top.enter_context(tc.tile_pool(name="dram", bufs=1, space="DRAM"))
        a2a_in = dramp.tile([NC_, NB, RPC], BF16)
        a2a_out = dramp.tile([NC_, NB, RPC], BF16)
        ctx_b = dramp.tile([128, NB * 32], BF16)      # [d, slot, head-block]
        agc = dramp.tile([NC_, 128, NB * 32], BF16, addr_space="Shared")
        hsl_b = dramp.tile([B, 512], F32)
        agh = dramp.tile([B * 8, 512], F32, addr_space="Shared")
        moe_ba = dramp.tile([B, 2048], BF16)
        moe_bb = dramp.tile([B, 2048], BF16)
        ar_oa = dramp.tile([B, 2048], BF16, addr_space="Shared")
        ar_ob = dramp.tile([B, 2048], BF16, addr_space="Shared")

        const = top.enter_context(tc.tile_pool(name="const", bufs=1))
        id64b = const.tile([64, 64], BF16)
        make_identity(nc, id64b)
        id64f = const.tile([64, 64], F32)
        make_identity(nc, id64f)
        id32b = const.tile([32, 32], BF16)
        make_identity(nc, id32b)
        warm = const.tile([64, 512], BF16)
        nc.gpsimd.memset(warm[:], 0.0)
        zero_col = const.tile([128, 1], F32)
        nc.gpsimd.memset(zero_col[:], 0.0)
        eps_col = const.tile([128, 1], F32)
        nc.gpsimd.memset(eps_col[:], float(HID) * EPS)
        nc.const_aps.aps[(F32, 0.0)] = zero_col[:]
        nc.const_aps.aps[(F32, float(HID) * EPS)] = eps_col[:]
        sel_bc = const.tile([64, 8], F32)
        nc.gpsimd.dma_start(sel_bc[:], sel_i.ap().to_broadcast((64, 8)))

        # long-lived activations
        acts = top.enter_context(tc.tile_pool(name="acts", bufs=1))
        scratch = acts.tile([B, HID], F32)       # square scratch / final out
        hid_full = acts.tile([B, HID], F32)      # post-attn residual hidden
        xT = acts.tile([128, 32 * 64], BF16)     # post-attn hidden transposed
        midT = acts.tile([128, 11 * 64], BF16)
        small = top.enter_context(tc.tile_pool(name="small", bufs=1))

        # ---------------- Phase A: norm1 -> qkv -> A2A ----------------
        with ExitStack() as pa:
            sA = pa.enter_context(tc.tile_pool(name="sA", bufs=1))
            qkvps = pa.enter_context(tc.tile_pool(name="qkvps", bufs=1, space="PSUM"))

            hT = sA.tile([128, 32 * 64], BF16)
            nc.scalar.dma_start(hT[:], hidT_i[:])
            h_sb = sA.tile([B, HID], F32)
            nc.scalar.dma_start(h_sb[:], hid_i[:])
            wq = sA.tile([128, 32, RPC], BF16)
            nc.scalar.dma_start(wq[:, :16], wqkvT_i[:, :16])
            nc.sync.dma_start(wq[:, 16:], wqkvT_i[:, 16:])

            ssq = small.tile([64, 1], F32, name="ssq")
            nc.scalar.activation(scratch[:], h_sb[:], AF.Square, accum_out=ssq[:])
            rs_col = small.tile([64, 1], F32, name="rs_col")
            nc.scalar.activation(rs_col[:], ssq[:], AF.Sqrt, bias=float(HID) * EPS)
            nc.vector.reciprocal(rs_col[:], rs_col[:])

            q1 = qkvps.tile([64, 512], F32, name="q1")
            q2 = qkvps.tile([64, 16], F32, name="q2")
            # PE p-state warmup: ~3us of dummy matmuls so the real qkv
            # matmuls run at max clock (no input dependencies).
            for _ in range(14):
                nc.tensor.matmul(q1[:], id64b[:], warm[:], start=True, stop=True)
            for k in range(32):
                nc.tensor.matmul(q1[:], hT[:, k * 64:(k + 1) * 64],
                                 wq[:, k, :512], start=(k == 0), stop=(k == 31))
                nc.tensor.matmul(q2[:], hT[:, k * 64:(k + 1) * 64],
                                 wq[:, k, 512:RPC], start=(k == 0), stop=(k == 31))
            qkv_sb = sA.tile([64, RPC], BF16)
            nc.vector.tensor_scalar_mul(qkv_sb[:, :512], q1[:], rs_col[:])
            nc.vector.tensor_scalar_mul(qkv_sb[:, 512:RPC], q2[:], rs_col[:])

            nc.scalar.dma_start(
                a2a_in[:].rearrange("s j x -> (s j) x"), qkv_sb[:])

            nc.gpsimd.collective_compute(
                "AllToAll", ALU.bypass, replica_groups=rg,
                ins=[a2a_in.opt()], outs=[a2a_out.opt()],
            )

        # ------- bulk weight streams (SP + Pool queues, consumption order) ----
        # w1 even chunks stream on SP from t~0; odd chunks are emitted on ACT
        # at the end of phase C (after its latency-critical reads) so the two
        # queues feed the E1 matmuls in parallel without blocking anything
        # E1 itself depends on.
        w1pe = top.enter_context(tc.tile_pool(name="w1pe", bufs=5))
        w1_tiles = {}
        for k in range(0, 32, 2):
            w1t = w1pe.tile([128, TWO_MI], BF16, name="w1te", tag="w1e")
            nc.sync.dma_start(w1t[:], w1T_i[k * 128:(k + 1) * 128, :])
            w1_tiles[k] = w1t
        wop = top.enter_context(tc.tile_pool(name="wop", bufs=1))
        wo_t = wop.tile([128, 32, 512], BF16)
        nc.gpsimd.dma_start(wo_t[:], woT_i[:])
        gwp = top.enter_context(tc.tile_pool(name="gwp", bufs=1))
        gw_t = gwp.tile([128, 32, 8], BF16)
        nc.gpsimd.dma_start(gw_t[:], gateT_i[:])
        # ---------------- Phase B: attention (8 slots) ----------------
        with ExitStack() as pb:
            sB = pb.enter_context(tc.tile_pool(name="sB", bufs=1))
            kvp = pb.enter_context(tc.tile_pool(name="kvp", bufs=2))
            ppp = pb.enter_context(tc.tile_pool(name="ppp", bufs=2))
            smb = pb.enter_context(tc.tile_pool(name="smb", bufs=2))
            scps = pb.enter_context(tc.tile_pool(name="scps", bufs=2, space="PSUM"))
            curps = pb.enter_context(tc.tile_pool(name="curps", bufs=2, space="PSUM"))
            ctxps = pb.enter_context(tc.tile_pool(name="ctxps", bufs=2, space="PSUM"))

            # qcols[(r*8+j), x] = source core r's qkv row x for my token j
            qcols = sB.tile([64, RPC], BF16)
            nc.scalar.dma_start(
                qcols[:], a2a_out[:].rearrange("r j x -> (r j) x"))
            # tpA[p, jt, r*8+j]: q for head-block h' = jt*8+r (head r*4+jt)
            tpA = sB.tile([128, 4, 64], BF16)
            for jt in range(4):
                pt = curps.tile([128, 64], BF16, name="ptB_q", tag="cur")
                nc.tensor.transpose(pt[:], qcols[:, jt * 128:(jt + 1) * 128],
                                    id64b[:])
                nc.vector.tensor_copy(tpA[:, jt, :], pt[:])
            kcurT = sB.tile([128, NB], BF16)
            vcur = sB.tile([1, NB * 129], BF16)
            for r in range(NC_):
                kblk = a2a_out[r, :, 512:RPC]           # (8 tokens, 16 d)
                nc.scalar.dma_start(
                    kcurT[r * QR_K:(r + 1) * QR_K, :],
                    kblk.rearrange("j t -> t j"))
                nc.scalar.dma_start(
                    vcur[:].rearrange("o (j x) -> o j x", x=129)
                    [:, :, r * QR_K:(r + 1) * QR_K],
                    kblk)
            nc.gpsimd.memset(
                vcur[:].rearrange("o (j x) -> o j x", x=129)[:, :, 128:129], 1.0)

            for j in range(NB):
                Cj = C[j]
                qT_b = (tpA[:].rearrange("p q (r i) -> p q r i", i=8)
                        [:, :, :, j])
                if Cj > 0:
                    kT_sb = kvp.tile([128, Cj * 128], BF16, name="kT_sb", tag="kT")
                    nc.scalar.dma_start(
                        kT_sb[:], kT_i[:, koff[j] * 128:(koff[j] + Cj) * 128])
                    v_sb = kvp.tile([128, Cj, 129], BF16, name="v_sb", tag="v")
                    nc.gpsimd.dma_start(
                        v_sb[:],
                        v_i[:, koff[j] * 129:(koff[j] + Cj) * 129]
                        .rearrange("p (c d) -> p c d", d=129))

                    sc = scps.tile([128, Cj * 32], F32, name="sc", tag="sc")
                    for c in range(Cj):
                        nc.tensor.matmul(sc[:, c * 32:(c + 1) * 32],
                                         kT_sb[:, c * 128:(c + 1) * 128],
                                         qT_b, start=True, stop=True)
                    pp = ppp.tile([128, Cj * 32], BF16, name="pp", tag="pp")
                    for c0 in range(0, Cj, 16):
                        ce = min(c0 + 16, Cj)
                        nc.scalar.activation(pp[:, c0 * 32:ce * 32],
                                             sc[:, c0 * 32:ce * 32], AF.Exp)

                cur = curps.tile([1, 32], F32, name="cur", tag="cur")
                nc.tensor.matmul(cur[:], kcurT[:, j:j + 1], qT_b,
                                 start=True, stop=True)
                pcur = smb.tile([1, 32], BF16, name="pcur", tag="pcur")
                nc.scalar.activation(pcur[:], cur[:], AF.Exp)

                ctx = ctxps.tile([32, 129], F32, name="ctx", tag="ctx")
                for c in range(Cj):
                    nc.tensor.matmul(ctx[:], pp[:, c * 32:(c + 1) * 32],
                                     v_sb[:, c, :], start=(c == 0), stop=False)
                nc.tensor.matmul(ctx[:], pcur[:],
                                 vcur[:, j * 129:(j + 1) * 129],
                                 start=(Cj == 0), stop=True)

                rden = smb.tile([32, 1], F32, name="rden", tag="rden")
                nc.vector.reciprocal(rden[:], ctx[:, 128:129])
                ctn_sb = smb.tile([32, 128], BF16, name="ctn_sb", tag="ctn")
                nc.vector.tensor_scalar_mul(ctn_sb[:], ctx[:, :128], rden[:])
                # transpose to (d, h) so the post-AG read needs no PE work
                ptc = curps.tile([128, 32], BF16, name="ptc", tag="cur")
                nc.tensor.transpose(ptc[:], ctn_sb[:], id32b[:])
                ctnT = smb.tile([128, 32], BF16, name="ctnT", tag="ctnT")
                nc.vector.tensor_copy(ctnT[:], ptc[:])
                nc.scalar.dma_start(ctx_b[:, j * 32:(j + 1) * 32], ctnT[:])

            nc.gpsimd.collective_compute(
                "AllGather", ALU.bypass, replica_groups=rg,
                ins=[ctx_b.opt()], outs=[agc.opt()],
            )

        # ---------------- Phase C: AG ctx -> Wo -> residual -> AG hidden ------
        with ExitStack() as pc:
            sC = pc.enter_context(tc.tile_pool(name="sC", bufs=1))
            wops = pc.enter_context(tc.tile_pool(name="wops", bufs=1, space="PSUM"))

            hidc = sC.tile([64, 512], F32)
            nc.scalar.dma_start(hidc[:], hidc_i[:])
            ctxA = sC.tile([128, 64, 32], BF16)      # [d, token, head-block]
            for r in range(NC_):
                nc.scalar.dma_start(
                    ctxA[:, r * 8:(r + 1) * 8, :],
                    agc[r].rearrange("p (j h) -> p j h", h=32))

            wo_ps = wops.tile([64, 512], F32)
            for k in range(32):
                nc.tensor.matmul(wo_ps[:], ctxA[:, :, k],
                                 wo_t[:, k, :], start=(k == 0), stop=(k == 31))
            hsl = sC.tile([64, 512], F32)
            nc.vector.tensor_tensor(hsl[:], wo_ps[:], hidc[:], op=ALU.add)
            nc.scalar.dma_start(hsl_b[:], hsl[:])
            nc.gpsimd.collective_compute(
                "AllGather", ALU.bypass, replica_groups=rg,
                ins=[hsl_b.opt()], outs=[agh.opt()],
            )
            nc.scalar.dma_start(hid_full[:].rearrange("b (r o) -> b r o", r=8),
                                agh.rearrange("(r b) o -> b r o", b=64))

        # odd w1 chunks on the ACT queue -- emitted only now so they sit
        # behind phase C's latency-critical reads, never ahead of them
        w1po = top.enter_context(tc.tile_pool(name="w1po", bufs=5))
        for k in range(1, 32, 2):
            w1t = w1po.tile([128, TWO_MI], BF16, name="w1to", tag="w1o")
            nc.scalar.dma_start(w1t[:], w1T_i[k * 128:(k + 1) * 128, :])
            w1_tiles[k] = w1t
        # w2 stream (Pool queue; emitted after the AG2 trigger so a pool
        # stall can never block a collective the MoE phases depend on)
        w2p = top.enter_context(tc.tile_pool(name="w2p", bufs=12))
        w2_tiles = []
        for w2x_i in (w2aT_i, w2bT_i):
            for mk in range(11):
                w2t = w2p.tile([128, 2048], BF16, name="w2t", tag="w2")
                nc.gpsimd.dma_start(w2t[:], w2x_i[mk * 128:(mk + 1) * 128, :])
                w2_tiles.append(w2t)

        # ------- Phase D/E1: xT transposes + w1 + gate (interleaved) ----------
        wsel_col = small.tile([64, 1], F32, name="wsel_col")
        rs2 = small.tile([64, 1], F32, name="rs2")
        with ExitStack() as pe1:
            sD = pe1.enter_context(tc.tile_pool(name="sD", bufs=1))
            ptD = pe1.enter_context(tc.tile_pool(name="ptD", bufs=1, space="PSUM"))
            gups = pe1.enter_context(tc.tile_pool(name="gups", bufs=1, space="PSUM"))
            gps = pe1.enter_context(tc.tile_pool(name="gps", bufs=1, space="PSUM"))

            ssq2 = small.tile([64, 1], F32, name="ssq2")
            nc.scalar.activation(scratch[:], hid_full[:], AF.Square,
                                 accum_out=ssq2[:])
            nc.scalar.activation(rs2[:], ssq2[:], AF.Sqrt, bias=float(HID) * EPS)
            nc.vector.reciprocal(rs2[:], rs2[:])

            gu = gups.tile([64, TWO_MI], F32)
            gpst = gps.tile([64, 8], F32)
            g_ps = gpst[:]
            slices = [(o * 512, min(512, TWO_MI - o * 512)) for o in range(6)]
            for k in range(32):
                pt = ptD.tile([128, 64], F32, name="ptD_t", tag="ptD_t")
                nc.tensor.transpose(pt[:], hid_full[:, k * 128:(k + 1) * 128],
                                    id64f[:])
                nc.vector.tensor_copy(xT[:, k * 64:(k + 1) * 64], pt[:])
                w1t = w1_tiles[k]
                for (off, w) in slices:
                    nc.tensor.matmul(gu[:, off:off + w],
                                     xT[:, k * 64:(k + 1) * 64],
                                     w1t[:, off:off + w],
                                     start=(k == 0), stop=(k == 31))
                nc.tensor.matmul(g_ps, xT[:, k * 64:(k + 1) * 64],
                                 gw_t[:, k, :], start=(k == 0), stop=(k == 31))

            gu_s = sD.tile([64, TWO_MI], BF16)
            nc.vector.tensor_scalar_mul(gu_s[:], gu[:], rs2[:])
            sg = sD.tile([64, MI], BF16)
            nc.scalar.activation(sg[:], gu_s[:, :MI], AF.Silu)
            mid = sD.tile([64, MI], BF16)
            nc.vector.tensor_tensor(mid[:], sg[:], gu_s[:, MI:], op=ALU.mult)

            for mk in range(11):
                pt = ptD.tile([128, 64], BF16, name="ptE_t", tag="ptD_t")
                nc.tensor.transpose(pt[:], mid[:, mk * 128:(mk + 1) * 128],
                                    id64b[:])
                nc.vector.tensor_copy(midT[:, mk * 64:(mk + 1) * 64], pt[:])

            # gate softmax + top-2 + renormalize + per-core select (fp32)
            pg = sD.tile([64, 8], F32)
            nc.scalar.activation(pg[:], g_ps, AF.Exp, scale=rs2[:])
            m1c = sD.tile([64, 1], F32)
            nc.vector.reduce_max(m1c[:], pg[:], axis=mybir.AxisListType.X)
            eq1 = sD.tile([64, 8], F32)
            nc.vector.tensor_scalar(eq1[:], pg[:], m1c[:], None, op0=ALU.is_ge)
            t1 = sD.tile([64, 8], F32)
            nc.vector.tensor_tensor(t1[:], pg[:], eq1[:], op=ALU.mult)
            nc.vector.tensor_tensor(t1[:], pg[:], t1[:], op=ALU.subtract)
            m2c = sD.tile([64, 1], F32)
            nc.vector.reduce_max(m2c[:], t1[:], axis=mybir.AxisListType.X)
            keep = sD.tile([64, 8], F32)
            nc.vector.tensor_scalar(keep[:], pg[:], m2c[:], None, op0=ALU.is_ge)
            wsum = sD.tile([64, 1], F32)
            nc.vector.tensor_tensor(wsum[:], m1c[:], m2c[:], op=ALU.add)
            nc.vector.reciprocal(wsum[:], wsum[:])
            wts = sD.tile([64, 8], F32)
            nc.vector.tensor_tensor(wts[:], pg[:], keep[:], op=ALU.mult)
            nc.vector.tensor_scalar_mul(wts[:], wts[:], wsum[:])
            nc.vector.tensor_tensor(wts[:], wts[:], sel_bc[:], op=ALU.mult)
            nc.vector.reduce_sum(wsel_col[:], wts[:], axis=mybir.AxisListType.X)

        # ------ Phase E2: w2 (two pipelined halves) + combine + AllReduce ----
        with ExitStack() as pe2:
            mops = pe2.enter_context(tc.tile_pool(name="mops", bufs=2, space="PSUM"))
            sF = pe2.enter_context(tc.tile_pool(name="sF", bufs=1))
            mo0 = mops.tile([64, 2048], F32, name="mo0", tag="mo")
            mo1 = mops.tile([64, 2048], F32, name="mo1", tag="mo")
            for half, (mo, moe_x, ar_x) in enumerate(
                    ((mo0, moe_ba, ar_oa), (mo1, moe_bb, ar_ob))):
                for mk in range(11):
                    w2t = w2_tiles[half * 11 + mk]
                    for oc in range(4):
                        nc.tensor.matmul(mo[:, oc * 512:(oc + 1) * 512],
                                         midT[:, mk * 64:(mk + 1) * 64],
                                         w2t[:, oc * 512:(oc + 1) * 512],
                                         start=(mk == 0), stop=(mk == 10))
                moe_sb = sF.tile([64, 2048], BF16, name="moe_sb", tag="moes",
                                 bufs=2)
                nc.vector.tensor_scalar_mul(moe_sb[:], mo[:], wsel_col[:])
                nc.scalar.dma_start(moe_x[:], moe_sb[:])
                nc.gpsimd.collective_compute(
                    "AllReduce", ALU.add, replica_groups=rg,
                    ins=[moe_x.opt()], outs=[ar_x.opt()],
                )
                ar_sb = sF.tile([B, 2048], BF16, name="ar_sb", tag="ars",
                                bufs=2)
                nc.scalar.dma_start(ar_sb[:], ar_x[:])
                cs = slice(half * 2048, half * 2048 + 2048)
                nc.vector.tensor_tensor(scratch[:, cs], ar_sb[:],
                                        hid_full[:, cs], op=ALU.add)
                nc.scalar.dma_start(out_o[:, cs], scratch[:, cs])

    nc.compile()
    return nc


_NC_CACHE = {}


def _get_program(C):
    if C not in _NC_CACHE:
        _NC_CACHE[C] = _build_program(C)
    return _NC_CACHE[C]


def kernel(hidden_states, positions, k_cache, v_cache, seq_lens,
           norm1_w, norm2_w, Wqkv, Wo, gate_w, w1, w2):
    global LAST_RESULT
    sl = np.asarray(seq_lens, np.int64)
    perm, C = _plan(sl)
    nc = _get_program(C)
    CSUM = sum(C)
    koff = np.concatenate([[0], np.cumsum(C)]).astype(int)

    hs_all = np.asarray(hidden_states, np.float32).reshape(B, HID)
    hs = hs_all[perm]                                # permuted token order
    scale = np.float32(HD) ** -0.5
    n1 = np.asarray(norm1_w, np.float32) * 64.0
    n2 = np.asarray(norm2_w, np.float32) * 64.0

    wq_full = np.asarray(Wqkv, np.float32)
    # hidT[p, k*64+b] = hs[b, 128k+p]
    hidT = np.ascontiguousarray(
        hs.T.reshape(32, 128, 64).transpose(1, 0, 2).reshape(128, 32 * 64)
    ).astype(BF)

    kc = np.asarray(k_cache, np.float32)
    vc = np.asarray(v_cache, np.float32)
    gT = (np.asarray(gate_w, np.float32) * n2[None, :]).T  # (4096, 8)
    gTt = np.ascontiguousarray(gT.reshape(32, 128, 8).transpose(1, 0, 2)).astype(BF)
    Wo_f = np.asarray(Wo, np.float32)
    w1_f = np.asarray(w1, np.float32)
    w2_f = np.asarray(w2, np.float32)

    in_maps = []
    for c in range(NC_):
        # qkv rows for this core: q rows [c*512,(c+1)*512) (scaled) + k rows
        rows = np.concatenate([
            wq_full[c * QR_Q:(c + 1) * QR_Q] * scale,
            wq_full[NH * HD + c * QR_K: NH * HD + (c + 1) * QR_K],
        ]) * n1[None, :]                              # (528, 4096)
        wqkvT = np.ascontiguousarray(
            rows.T.reshape(32, 128, RPC).transpose(1, 0, 2)).astype(BF)

        # on-chip ctx head-block order h' -> actual head (h'%8)*4 + h'//8
        perm_h = np.array([(k % 8) * 4 + k // 8 for k in range(32)])
        woT = np.ascontiguousarray(
            Wo_f[c * 512:(c + 1) * 512].T.reshape(32, 128, 512)[perm_h]
            .transpose(1, 0, 2)).astype(BF)

        kT_buf = np.zeros((128, max(CSUM, 1) * 128), BF)
        v_buf = np.zeros((128, max(CSUM, 1) * 129), BF)
        for j in range(NB):
            Cj = C[j]
            if Cj == 0:
                continue
            b = perm[c * NB + j]
            n_real = int(sl[b]) - 1                   # positions [0, sl-1)
            span = Cj * 128
            kchunk = np.zeros((span, HD), np.float32)
            kchunk[:n_real] = kc[b, :n_real]
            kT_buf[:, koff[j] * 128: koff[j] * 128 + span] = \
                kchunk.T.astype(BF)
            vchunk = np.zeros((Cj, 128, 129), np.float32)
            vflat = vchunk.reshape(span, 129)
            vflat[:n_real, :HD] = vc[b, :n_real]
            vflat[:n_real, HD] = 1.0
            v_buf[:, koff[j] * 129: (koff[j] + Cj) * 129] = \
                vchunk.transpose(1, 0, 2).reshape(128, Cj * 129).astype(BF)

        sel = np.zeros((1, 8), np.float32)
        sel[0, c] = 1.0
        in_maps.append({
            "hid": hs,
            "hidT": hidT,
            "hidcols": np.ascontiguousarray(hs[:, c * 512:(c + 1) * 512]),
            "wqkvT": wqkvT,
            "woT": woT,
            "gateT": gTt,
            "w1T": np.ascontiguousarray((w1_f[c] * n2[None, :]).T).astype(BF),
            "w2aT": np.ascontiguousarray(w2_f[c, :2048].T).astype(BF),
            "w2bT": np.ascontiguousarray(w2_f[c, 2048:].T).astype(BF),
            "kT": kT_buf,
            "v": v_buf,
            "sel": sel,
        })

    LAST_RESULT = run_bass_kernel_spmd(nc, in_maps, core_ids=list(range(NC_)))
    res = LAST_RESULT.results[0]["out"]               # (64, 4096), permuted
    out = np.empty((B, HID), np.float32)
    out[perm] = res
    return out.reshape(B, 1, HID).astype(np.float32)


# revision 49
# speedup vs baseline: 2.0812x; 1.0192x over previous
"""DeepSeek-V2 decode layer on 8 TRN2 NeuronCores (Bass/Tile SPMD kernel).

v2 design (bf16 + seq-truncated attention + balanced batch placement):
  - All matmul operands bf16 (fp32 PSUM accumulate); residual stream, softmax
    denominators, and gate top-2 stay fp32.  bf16-everything sim rel-err vs
    the fp32 reference is ~2.4e-3 (budget 2e-2).
  - QKV proj row-parallel (512 q rows + 16 current-k rows per core), AllToAll
    (bf16) redistributes q^T/kcur^T to batch-sharded layout.
  - Attention data-parallel: 8 sequences per core, chosen by LPT bin-packing
    on ceil((seq_len-1)/128) so all cores get the same per-slot chunk budget
    C_j (required for SPMD) with minimal padding.  Host zero-pads K^T columns
    and V rows outside [0, seq_len-1); V carries a ones-column so the softmax
    denominator falls out of the ctx matmul for free.  The current token's
    k==v vector is applied via tiny rank-1 matmuls (uniform across cores).
  - ctx matmul uses p-chunks as stationary so ctx lands directly as
    (head, dim) -- no per-batch transpose, no DRAM broadcast round-trip.
  - Wo output-column-parallel (512 cols/core) after bf16 AllGather of ctx;
    post-attn hidden AllGather in fp32 (residual precision).
  - MoE expert-parallel (1 expert/core), norm2 weights folded into w1/gate
    host-side and the 1/rms per-token scale applied on gu -- so w1 matmuls
    start right after the hidden AllGather without waiting on the rsqrt.
    Expert outputs combined via bf16 AllReduce.
  - Bulk weight DMA spread across the SP (w1) and Pool (wo, gate, w2) HWDGE
    queues; latency-critical per-phase traffic rides ACT and DVE queues.
"""

import os
import sys

import numpy as np

for _p in ("/opt/trn_rl_repo", "/root/.axon_site/_ro/trn_rl_repo", "/root/.axon_site"):
    if _p not in sys.path and os.path.isdir(_p):
        sys.path.append(_p)


def _ensure_ntff_hook():
    """This image's antenv lacks axon_hooks; shim it so BASS_TRACE works."""
    import types

    try:
        import antenv.axon_hooks  # noqa: F401
        return
    except ImportError:
        pass
    import antenv

    mod = types.ModuleType("antenv.axon_hooks")
    _state = {"h": None}
    mod.set_axon_ntff_profile_hook = lambda h: _state.__setitem__("h", h)
    mod.get_axon_ntff_profile_hook = lambda: _state["h"]
    sys.modules["antenv.axon_hooks"] = mod
    antenv.axon_hooks = mod
    try:
        sys.path.insert(0, "/root/.axon_site/trn_agent_boot")
        import trn_boot

        so_path = "/opt/axon/libaxon_pjrt.so"
        if os.path.exists(so_path):
            mod.set_axon_ntff_profile_hook(
                trn_boot._ntff_profile_via_ctypes(so_path))
    except Exception as e:  # tracing degrades; compile+run still work
        print(f"ntff hook install failed: {e}")


_ensure_ntff_hook()

import ml_dtypes
import concourse.bacc as bacc
import concourse.bass as bass
import concourse.mybir as mybir
import concourse.tile as tile
from concourse.bass_utils import run_bass_kernel_spmd
from concourse.masks import make_identity
from contextlib import ExitStack

F32 = mybir.dt.float32
BF16 = mybir.dt.bfloat16
AF = mybir.ActivationFunctionType
ALU = mybir.AluOpType
BF = ml_dtypes.bfloat16

B, HID, S, NH, HD = 64, 4096, 4096, 32, 128
MI, TWO_MI = 1408, 2816
NC_ = 8
NB = B // NC_                 # 8 local batches (slots) per core
QR_Q, QR_K = 512, 16          # per-core q rows / current-k rows of Wqkv
RPC = QR_Q + QR_K             # 528
EPS = 1e-6

LAST_RESULT = None


def _plan(seq_lens):
    """Slot budgets + batch->core placement balanced on KV chunk count.

    Real attended positions per batch are [0, sl-1) (the current token is
    handled separately), so cb = ceil((sl-1)/128).  Sort desc, group ranks
    [8j, 8j+8) into slot j with budget C[j] = group max; core c takes the
    c-th member of each group.  perm[c*8+j] = global batch index.
    """
    sl = np.asarray(seq_lens, np.int64)
    cb = np.maximum(sl - 1, 0)
    cb = -(-cb // 128)
    order = np.argsort(-cb, kind="stable")
    C = [int(cb[order[j * 8]]) for j in range(NB)]
    perm = np.empty(B, np.int64)
    for j in range(NB):
        for c in range(NC_):
            perm[c * NB + j] = order[j * 8 + c]
    return perm, tuple(C)


def _build_program(C):
    nc = bacc.Bacc(None, target_bir_lowering=False, num_devices=NC_)

    CSUM = sum(C)
    koff = np.concatenate([[0], np.cumsum(C)]).astype(int)  # chunk offsets

    hid_i = nc.dram_tensor("hid", [B, HID], F32, kind="ExternalInput")
    hidT_i = nc.dram_tensor("hidT", [128, 32 * 64], BF16, kind="ExternalInput")
    hidc_i = nc.dram_tensor("hidcols", [B, 512], F32, kind="ExternalInput")
    wqkvT_i = nc.dram_tensor("wqkvT", [128, 32, RPC], BF16, kind="ExternalInput")
    woT_i = nc.dram_tensor("woT", [128, 32, 512], BF16, kind="ExternalInput")
    gateT_i = nc.dram_tensor("gateT", [128, 32, 8], BF16, kind="ExternalInput")
    w1T_i = nc.dram_tensor("w1T", [HID, TWO_MI], BF16, kind="ExternalInput")
    w2aT_i = nc.dram_tensor("w2aT", [MI, 2048], BF16, kind="ExternalInput")
    w2bT_i = nc.dram_tensor("w2bT", [MI, 2048], BF16, kind="ExternalInput")
    kT_i = nc.dram_tensor("kT", [128, max(CSUM, 1) * 128], BF16,
                          kind="ExternalInput")
    v_i = nc.dram_tensor("v", [128, max(CSUM, 1) * 129], BF16,
                         kind="ExternalInput")
    sel_i = nc.dram_tensor("sel", [1, 8], F32, kind="ExternalInput")
    out_o = nc.dram_tensor("out", [B, HID], F32, kind="ExternalOutput")

    rg = [list(range(NC_))]

    with tile.TileContext(nc) as tc, ExitStack() as top:
        # A2A payload is token-major: block for dest core s = qkv_sb rows
        # [s*8,(s+1)*8) -- a single contiguous SBUF->DRAM copy.  The q
        # transpose happens consumer-side (4 PE transposes); the resulting
        # head-block order h' = jt*8 + r (actual head r*4+jt) is absorbed
        # into the host-side layout of Wo.
        dramp = top.enter_context(tc.tile_pool(name="dram", bufs=1, space="DRAM"))
        a2a_in = dramp.tile([NC_, NB, RPC], BF16)
        a2a_out = dramp.tile([NC_, NB, RPC], BF16)
        ctx_b = dramp.tile([128, NB * 32], BF16)      # [d, slot, head-block]
        agc = dramp.tile([NC_, 128, NB * 32], BF16, addr_space="Shared")
        hsl_b = dramp.tile([B, 512], F32)
        agh = dramp.tile([B * 8, 512], F32, addr_space="Shared")
        moe_ba = dramp.tile([B, 2048], BF16)
        moe_bb = dramp.tile([B, 2048], BF16)
        ar_oa = dramp.tile([B, 2048], BF16, addr_space="Shared")
        ar_ob = dramp.tile([B, 2048], BF16, addr_space="Shared")

        const = top.enter_context(tc.tile_pool(name="const", bufs=1))
        id64b = const.tile([64, 64], BF16)
        make_identity(nc, id64b)
        id64f = const.tile([64, 64], F32)
        make_identity(nc, id64f)
        id32b = const.tile([32, 32], BF16)
        make_identity(nc, id32b)
        warm = const.tile([64, 512], BF16)
        nc.gpsimd.memset(warm[:], 0.0)
        zero_col = const.tile([128, 1], F32)
        nc.gpsimd.memset(zero_col[:], 0.0)
        eps_col = const.tile([128, 1], F32)
        nc.gpsimd.memset(eps_col[:], float(HID) * EPS)
        nc.const_aps.aps[(F32, 0.0)] = zero_col[:]
        nc.const_aps.aps[(F32, float(HID) * EPS)] = eps_col[:]
        sel_bc = const.tile([64, 8], F32)
        nc.gpsimd.dma_start(sel_bc[:], sel_i.ap().to_broadcast((64, 8)))

        # long-lived activations
        acts = top.enter_context(tc.tile_pool(name="acts", bufs=1))
        scratch = acts.tile([B, HID], F32)       # square scratch / final out
        hid_full = acts.tile([B, HID], F32)      # post-attn residual hidden
        xT = acts.tile([128, 32 * 64], BF16)     # post-attn hidden transposed
        midT = acts.tile([128, 11 * 64], BF16)
        small = top.enter_context(tc.tile_pool(name="small", bufs=1))

        # ---------------- Phase A: norm1 -> qkv -> A2A ----------------
        with ExitStack() as pa:
            sA = pa.enter_context(tc.tile_pool(name="sA", bufs=1))
            qkvps = pa.enter_context(tc.tile_pool(name="qkvps", bufs=1, space="PSUM"))

            hT = sA.tile([128, 32 * 64], BF16)
            nc.scalar.dma_start(hT[:], hidT_i[:])
            wq = sA.tile([128, 32, RPC], BF16)
            nc.scalar.dma_start(wq[:, :16], wqkvT_i[:, :16])
            nc.sync.dma_start(wq[:, 16:], wqkvT_i[:, 16:])
            h_sb = sA.tile([B, HID], F32)
            nc.scalar.dma_start(h_sb[:], hid_i[:])

            ssq = small.tile([64, 1], F32, name="ssq")
            nc.scalar.activation(scratch[:], h_sb[:], AF.Square, accum_out=ssq[:])
            rs_col = small.tile([64, 1], F32, name="rs_col")
            nc.scalar.activation(rs_col[:], ssq[:], AF.Sqrt, bias=float(HID) * EPS)
            nc.vector.reciprocal(rs_col[:], rs_col[:])

            q1 = qkvps.tile([64, 512], F32, name="q1")
            q2 = qkvps.tile([64, 16], F32, name="q2")
            # PE p-state warmup: ~3us of dummy matmuls so the real qkv
            # matmuls run at max clock (no input dependencies).
            for _ in range(14):
                nc.tensor.matmul(q1[:], id64b[:], warm[:], start=True, stop=True)
            for k in range(32):
                nc.tensor.matmul(q1[:], hT[:, k * 64:(k + 1) * 64],
                                 wq[:, k, :512], start=(k == 0), stop=(k == 31))
                nc.tensor.matmul(q2[:], hT[:, k * 64:(k + 1) * 64],
                                 wq[:, k, 512:RPC], start=(k == 0), stop=(k == 31))
            qkv_sb = sA.tile([64, RPC], BF16)
            nc.vector.tensor_scalar_mul(qkv_sb[:, :512], q1[:], rs_col[:])
            nc.vector.tensor_scalar_mul(qkv_sb[:, 512:RPC], q2[:], rs_col[:])

            nc.scalar.dma_start(
                a2a_in[:].rearrange("s j x -> (s j) x"), qkv_sb[:])

            nc.gpsimd.collective_compute(
                "AllToAll", ALU.bypass, replica_groups=rg,
                ins=[a2a_in.opt()], outs=[a2a_out.opt()],
            )

        # ------- bulk weight streams (SP + Pool queues, consumption order) ----
        # w1 k%3==0 chunks stream on SP from t~0; the other two thirds are
        # emitted on ACT and Pool after the phase C trigger points so the
        # three queues feed the E1 matmuls in parallel without blocking
        # anything E1 itself depends on.
        w1pe = top.enter_context(tc.tile_pool(name="w1pe", bufs=5))
        w1_tiles = {}
        for k in range(0, 32, 3):
            w1t = w1pe.tile([128, TWO_MI], BF16, name="w1te", tag="w1e")
            nc.sync.dma_start(w1t[:], w1T_i[k * 128:(k + 1) * 128, :])
            w1_tiles[k] = w1t
        wop = top.enter_context(tc.tile_pool(name="wop", bufs=1))
        wo_t = wop.tile([128, 32, 512], BF16)
        nc.gpsimd.dma_start(wo_t[:], woT_i[:])
        gwp = top.enter_context(tc.tile_pool(name="gwp", bufs=1))
        gw_t = gwp.tile([128, 32, 8], BF16)
        nc.gpsimd.dma_start(gw_t[:], gateT_i[:])
        # ---------------- Phase B: attention (8 slots) ----------------
        with ExitStack() as pb:
            sB = pb.enter_context(tc.tile_pool(name="sB", bufs=1))
            kvp = pb.enter_context(tc.tile_pool(name="kvp", bufs=2))
            ppp = pb.enter_context(tc.tile_pool(name="ppp", bufs=2))
            smb = pb.enter_context(tc.tile_pool(name="smb", bufs=2))
            scps = pb.enter_context(tc.tile_pool(name="scps", bufs=2, space="PSUM"))
            curps = pb.enter_context(tc.tile_pool(name="curps", bufs=2, space="PSUM"))
            ctxps = pb.enter_context(tc.tile_pool(name="ctxps", bufs=2, space="PSUM"))

            # qcols[(r*8+j), x] = source core r's qkv row x for my token j
            qcols = sB.tile([64, RPC], BF16)
            nc.scalar.dma_start(
                qcols[:], a2a_out[:].rearrange("r j x -> (r j) x"))
            # tpA[p, jt, r*8+j]: q for head-block h' = jt*8+r (head r*4+jt)
            tpA = sB.tile([128, 4, 64], BF16)
            for jt in range(4):
                pt = curps.tile([128, 64], BF16, name="ptB_q", tag="cur")
                nc.tensor.transpose(pt[:], qcols[:, jt * 128:(jt + 1) * 128],
                                    id64b[:])
                nc.vector.tensor_copy(tpA[:, jt, :], pt[:])
            kcurT = sB.tile([128, NB], BF16)
            vcur = sB.tile([1, NB * 129], BF16)
            for r in range(NC_):
                kblk = a2a_out[r, :, 512:RPC]           # (8 tokens, 16 d)
                nc.scalar.dma_start(
                    kcurT[r * QR_K:(r + 1) * QR_K, :],
                    kblk.rearrange("j t -> t j"))
                nc.scalar.dma_start(
                    vcur[:].rearrange("o (j x) -> o j x", x=129)
                    [:, :, r * QR_K:(r + 1) * QR_K],
                    kblk)
            nc.gpsimd.memset(
                vcur[:].rearrange("o (j x) -> o j x", x=129)[:, :, 128:129], 1.0)

            for j in range(NB):
                Cj = C[j]
                qT_b = (tpA[:].rearrange("p q (r i) -> p q r i", i=8)
                        [:, :, :, j])
                if Cj > 0:
                    kT_sb = kvp.tile([128, Cj * 128], BF16, name="kT_sb", tag="kT")
                    nc.scalar.dma_start(
                        kT_sb[:], kT_i[:, koff[j] * 128:(koff[j] + Cj) * 128])
                    v_sb = kvp.tile([128, Cj, 129], BF16, name="v_sb", tag="v")
                    nc.gpsimd.dma_start(
                        v_sb[:],
                        v_i[:, koff[j] * 129:(koff[j] + Cj) * 129]
                        .rearrange("p (c d) -> p c d", d=129))

                    sc = scps.tile([128, Cj * 32], F32, name="sc", tag="sc")
                    for c in range(Cj):
                        nc.tensor.matmul(sc[:, c * 32:(c + 1) * 32],
                                         kT_sb[:, c * 128:(c + 1) * 128],
                                         qT_b, start=True, stop=True)
                    pp = ppp.tile([128, Cj * 32], BF16, name="pp", tag="pp")
                    for c0 in range(0, Cj, 16):
                        ce = min(c0 + 16, Cj)
                        nc.scalar.activation(pp[:, c0 * 32:ce * 32],
                                             sc[:, c0 * 32:ce * 32], AF.Exp)

                cur = curps.tile([1, 32], F32, name="cur", tag="cur")
                nc.tensor.matmul(cur[:], kcurT[:, j:j + 1], qT_b,
                                 start=True, stop=True)
                pcur = smb.tile([1, 32], BF16, name="pcur", tag="pcur")
                nc.scalar.activation(pcur[:], cur[:], AF.Exp)

                ctx = ctxps.tile([32, 129], F32, name="ctx", tag="ctx")
                for c in range(Cj):
                    nc.tensor.matmul(ctx[:], pp[:, c * 32:(c + 1) * 32],
                                     v_sb[:, c, :], start=(c == 0), stop=False)
                nc.tensor.matmul(ctx[:], pcur[:],
                                 vcur[:, j * 129:(j + 1) * 129],
                                 start=(Cj == 0), stop=True)

                rden = smb.tile([32, 1], F32, name="rden", tag="rden")
                nc.vector.reciprocal(rden[:], ctx[:, 128:129])
                ctn_sb = smb.tile([32, 128], BF16, name="ctn_sb", tag="ctn")
                nc.vector.tensor_scalar_mul(ctn_sb[:], ctx[:, :128], rden[:])
                # transpose to (d, h) so the post-AG read needs no PE work
                ptc = curps.tile([128, 32], BF16, name="ptc", tag="cur")
                nc.tensor.transpose(ptc[:], ctn_sb[:], id32b[:])
                ctnT = smb.tile([128, 32], BF16, name="ctnT", tag="ctnT")
                nc.vector.tensor_copy(ctnT[:], ptc[:])
                nc.scalar.dma_start(ctx_b[:, j * 32:(j + 1) * 32], ctnT[:])

            nc.gpsimd.collective_compute(
                "AllGather", ALU.bypass, replica_groups=rg,
                ins=[ctx_b.opt()], outs=[agc.opt()],
            )

        # ---------------- Phase C: AG ctx -> Wo -> residual -> AG hidden ------
        with ExitStack() as pc:
            sC = pc.enter_context(tc.tile_pool(name="sC", bufs=1))
            wops = pc.enter_context(tc.tile_pool(name="wops", bufs=1, space="PSUM"))

            hidc = sC.tile([64, 512], F32)
            nc.scalar.dma_start(hidc[:], hidc_i[:])
            ctxA = sC.tile([128, 64, 32], BF16)      # [d, token, head-block]
            for r in range(NC_):
                nc.scalar.dma_start(
                    ctxA[:, r * 8:(r + 1) * 8, :],
                    agc[r].rearrange("p (j h) -> p j h", h=32))

            wo_ps = wops.tile([64, 512], F32)
            for k in range(32):
                nc.tensor.matmul(wo_ps[:], ctxA[:, :, k],
                                 wo_t[:, k, :], start=(k == 0), stop=(k == 31))
            hsl = sC.tile([64, 512], F32)
            nc.vector.tensor_tensor(hsl[:], wo_ps[:], hidc[:], op=ALU.add)
            nc.scalar.dma_start(hsl_b[:], hsl[:])
            nc.gpsimd.collective_compute(
                "AllGather", ALU.bypass, replica_groups=rg,
                ins=[hsl_b.opt()], outs=[agh.opt()],
            )
            nc.scalar.dma_start(hid_full[:].rearrange("b (r o) -> b r o", r=8),
                                agh.rearrange("(r b) o -> b r o", b=64))

        # remaining w1 thirds on the ACT and Pool queues -- emitted only now
        # so they sit behind phase C's latency-critical reads and the AG2
        # trigger, never ahead of them
        w1po = top.enter_context(tc.tile_pool(name="w1po", bufs=4))
        w1pg = top.enter_context(tc.tile_pool(name="w1pg", bufs=4))
        for k in range(32):
            if k % 3 == 0:
                continue
            if k % 3 == 1:
                w1t = w1po.tile([128, TWO_MI], BF16, name="w1to", tag="w1o")
                nc.scalar.dma_start(w1t[:], w1T_i[k * 128:(k + 1) * 128, :])
            else:
                w1t = w1pg.tile([128, TWO_MI], BF16, name="w1tg", tag="w1g")
                nc.gpsimd.dma_start(w1t[:], w1T_i[k * 128:(k + 1) * 128, :])
            w1_tiles[k] = w1t
        # w2 stream (Pool queue; emitted after the AG2 trigger so a pool
        # stall can never block a collective the MoE phases depend on)
        w2p = top.enter_context(tc.tile_pool(name="w2p", bufs=12))
        w2_tiles = []
        for w2x_i in (w2aT_i, w2bT_i):
            for mk in range(11):
                w2t = w2p.tile([128, 2048], BF16, name="w2t", tag="w2")
                nc.gpsimd.dma_start(w2t[:], w2x_i[mk * 128:(mk + 1) * 128, :])
                w2_tiles.append(w2t)

        # ------- Phase D/E1: xT transposes + w1 + gate (interleaved) ----------
        wsel_col = small.tile([64, 1], F32, name="wsel_col")
        rs2 = small.tile([64, 1], F32, name="rs2")
        with ExitStack() as pe1:
            sD = pe1.enter_context(tc.tile_pool(name="sD", bufs=1))
            ptD = pe1.enter_context(tc.tile_pool(name="ptD", bufs=1, space="PSUM"))
            gups = pe1.enter_context(tc.tile_pool(name="gups", bufs=1, space="PSUM"))
            gps = pe1.enter_context(tc.tile_pool(name="gps", bufs=1, space="PSUM"))

            ssq2 = small.tile([64, 1], F32, name="ssq2")
            nc.scalar.activation(scratch[:], hid_full[:], AF.Square,
                                 accum_out=ssq2[:])
            nc.scalar.activation(rs2[:], ssq2[:], AF.Sqrt, bias=float(HID) * EPS)
            nc.vector.reciprocal(rs2[:], rs2[:])

            gu = gups.tile([64, TWO_MI], F32)
            gpst = gps.tile([64, 8], F32)
            g_ps = gpst[:]
            slices = [(o * 512, min(512, TWO_MI - o * 512)) for o in range(6)]
            for k in range(32):
                pt = ptD.tile([128, 64], F32, name="ptD_t", tag="ptD_t")
                nc.tensor.transpose(pt[:], hid_full[:, k * 128:(k + 1) * 128],
                                    id64f[:])
                nc.vector.tensor_copy(xT[:, k * 64:(k + 1) * 64], pt[:])
                w1t = w1_tiles[k]
                for (off, w) in slices:
                    nc.tensor.matmul(gu[:, off:off + w],
                                     xT[:, k * 64:(k + 1) * 64],
                                     w1t[:, off:off + w],
                                     start=(k == 0), stop=(k == 31))
                nc.tensor.matmul(g_ps, xT[:, k * 64:(k + 1) * 64],
                                 gw_t[:, k, :], start=(k == 0), stop=(k == 31))

            gu_s = sD.tile([64, TWO_MI], BF16)
            nc.vector.tensor_scalar_mul(gu_s[:], gu[:], rs2[:])
            sg = sD.tile([64, MI], BF16)
            nc.scalar.activation(sg[:], gu_s[:, :MI], AF.Silu)
            mid = sD.tile([64, MI], BF16)
            nc.vector.tensor_tensor(mid[:], sg[:], gu_s[:, MI:], op=ALU.mult)

            for mk in range(11):
                pt = ptD.tile([128, 64], BF16, name="ptE_t", tag="ptD_t")
                nc.tensor.transpose(pt[:], mid[:, mk * 128:(mk + 1) * 128],
                                    id64b[:])
                nc.vector.tensor_copy(midT[:, mk * 64:(mk + 1) * 64], pt[:])

            # gate softmax + top-2 + renormalize + per-core select (fp32)
            pg = sD.tile([64, 8], F32)
            nc.scalar.activation(pg[:], g_ps, AF.Exp, scale=rs2[:])
            m1c = sD.tile([64, 1], F32)
            nc.vector.reduce_max(m1c[:], pg[:], axis=mybir.AxisListType.X)
            eq1 = sD.tile([64, 8], F32)
            nc.vector.tensor_scalar(eq1[:], pg[:], m1c[:], None, op0=ALU.is_ge)
            t1 = sD.tile([64, 8], F32)
            nc.vector.tensor_tensor(t1[:], pg[:], eq1[:], op=ALU.mult)
            nc.vector.tensor_tensor(t1[:], pg[:], t1[:], op=ALU.subtract)
            m2c = sD.tile([64, 1], F32)
            nc.vector.reduce_max(m2c[:], t1[:], axis=mybir.AxisListType.X)
            keep = sD.tile([64, 8], F32)
            nc.vector.tensor_scalar(keep[:], pg[:], m2c[:], None, op0=ALU.is_ge)
            wsum = sD.tile([64, 1], F32)
            nc.vector.tensor_tensor(wsum[:], m1c[:], m2c[:], op=ALU.add)
            nc.vector.reciprocal(wsum[:], wsum[:])
            wts = sD.tile([64, 8], F32)
            nc.vector.tensor_tensor(wts[:], pg[:], keep[:], op=ALU.mult)
            nc.vector.tensor_scalar_mul(wts[:], wts[:], wsum[:])
            nc.vector.tensor_tensor(wts[:], wts[:], sel_bc[:], op=ALU.mult)
            nc.vector.reduce_sum(wsel_col[:], wts[:], axis=mybir.AxisListType.X)

        # ------ Phase E2: w2 (two pipelined halves) + combine + AllReduce ----
        with ExitStack() as pe2:
            mops = pe2.enter_context(tc.tile_pool(name="mops", bufs=2, space="PSUM"))
            sF = pe2.enter_context(tc.tile_pool(name="sF", bufs=1))
            mo0 = mops.tile([64, 2048], F32, name="mo0", tag="mo")
            mo1 = mops.tile([64, 2048], F32, name="mo1", tag="mo")
            for half, (mo, moe_x, ar_x) in enumerate(
                    ((mo0, moe_ba, ar_oa), (mo1, moe_bb, ar_ob))):
                for mk in range(11):
                    w2t = w2_tiles[half * 11 + mk]
                    for oc in range(4):
                        nc.tensor.matmul(mo[:, oc * 512:(oc + 1) * 512],
                                         midT[:, mk * 64:(mk + 1) * 64],
                                         w2t[:, oc * 512:(oc + 1) * 512],
                                         start=(mk == 0), stop=(mk == 10))
                moe_sb = sF.tile([64, 2048], BF16, name="moe_sb", tag="moes",
                                 bufs=2)
                nc.vector.tensor_scalar_mul(moe_sb[:], mo[:], wsel_col[:])
                nc.scalar.dma_start(moe_x[:], moe_sb[:])
                nc.gpsimd.collective_compute(
                    "AllReduce", ALU.add, replica_groups=rg,
                    ins=[moe_x.opt()], outs=[ar_x.opt()],
                )
                ar_sb = sF.tile([B, 2048], BF16, name="ar_sb", tag="ars",
                                bufs=2)
                nc.scalar.dma_start(ar_sb[:], ar_x[:])
                cs = slice(half * 2048, half * 2048 + 2048)
                nc.vector.tensor_tensor(scratch[:, cs], ar_sb[:],
                                        hid_full[:, cs], op=ALU.add)
                nc.scalar.dma_start(out_o[:, cs], scratch[:, cs])

    nc.compile()
    return nc


_NC_CACHE = {}


def _get_program(C):
    if C not in _NC_CACHE:
        _NC_CACHE[C] = _build_program(C)
    return _NC_CACHE[C]


def kernel(hidden_states, positions, k_cache, v_cache, seq_lens,
           norm1_w, norm2_w, Wqkv, Wo, gate_w, w1, w2):
    global LAST_RESULT
    sl = np.asarray(seq_lens, np.int64)
    perm, C = _plan(sl)
    nc = _get_program(C)
    CSUM = sum(C)
    koff = np.concatenate([[0], np.cumsum(C)]).astype(int)

    hs_all = np.asarray(hidden_states, np.float32).reshape(B, HID)
    hs = hs_all[perm]                                # permuted token order
    scale = np.float32(HD) ** -0.5
    n1 = np.asarray(norm1_w, np.float32) * 64.0
    n2 = np.asarray(norm2_w, np.float32) * 64.0

    wq_full = np.asarray(Wqkv, np.float32)
    # hidT[p, k*64+b] = hs[b, 128k+p]
    hidT = np.ascontiguousarray(
        hs.T.reshape(32, 128, 64).transpose(1, 0, 2).reshape(128, 32 * 64)
    ).astype(BF)

    kc = np.asarray(k_cache, np.float32)
    vc = np.asarray(v_cache, np.float32)
    gT = (np.asarray(gate_w, np.float32) * n2[None, :]).T  # (4096, 8)
    gTt = np.ascontiguousarray(gT.reshape(32, 128, 8).transpose(1, 0, 2)).astype(BF)
    Wo_f = np.asarray(Wo, np.float32)
    w1_f = np.asarray(w1, np.float32)
    w2_f = np.asarray(w2, np.float32)

    in_maps = []
    for c in range(NC_):
        # qkv rows for this core: q rows [c*512,(c+1)*512) (scaled) + k rows
        rows = np.concatenate([
            wq_full[c * QR_Q:(c + 1) * QR_Q] * scale,
            wq_full[NH * HD + c * QR_K: NH * HD + (c + 1) * QR_K],
        ]) * n1[None, :]                              # (528, 4096)
        wqkvT = np.ascontiguousarray(
            rows.T.reshape(32, 128, RPC).transpose(1, 0, 2)).astype(BF)

        # on-chip ctx head-block order h' -> actual head (h'%8)*4 + h'//8
        perm_h = np.array([(k % 8) * 4 + k // 8 for k in range(32)])
        woT = np.ascontiguousarray(
            Wo_f[c * 512:(c + 1) * 512].T.reshape(32, 128, 512)[perm_h]
            .transpose(1, 0, 2)).astype(BF)

        kT_buf = np.zeros((128, max(CSUM, 1) * 128), BF)
        v_buf = np.zeros((128, max(CSUM, 1) * 129), BF)
        for j in range(NB):
            Cj = C[j]
            if Cj == 0:
                continue
            b = perm[c * NB + j]
            n_real = int(sl[b]) - 1                   # positions [0, sl-1)
            span = Cj * 128
            kchunk = np.zeros((span, HD), np.float32)
            kchunk[:n_real] = kc[b, :n_real]
            kT_buf[:, koff[j] * 128: koff[j] * 128 + span] = \
                kchunk.T.astype(BF)
            vchunk = np.zeros((Cj, 128, 129), np.float32)
            vflat = vchunk.reshape(span, 129)
            vflat[:n_real, :HD] = vc[b, :n_real]
            vflat[:n_real, HD] = 1.0
            v_buf[:, koff[j] * 129: (koff[j] + Cj) * 129] = \
                vchunk.transpose(1, 0, 2).reshape(128, Cj * 129).astype(BF)

        sel = np.zeros((1, 8), np.float32)
        sel[0, c] = 1.0
        in_maps.append({
            "hid": hs,
            "hidT": hidT,
            "hidcols": np.ascontiguousarray(hs[:, c * 512:(c + 1) * 512]),
            "wqkvT": wqkvT,
            "woT": woT,
            "gateT": gTt,
            "w1T": np.ascontiguousarray((w1_f[c] * n2[None, :]).T).astype(BF),
            "w2aT": np.ascontiguousarray(w2_f[c, :2048].T).astype(BF),
            "w2bT": np.ascontiguousarray(w2_f[c, 2048:].T).astype(BF),
            "kT": kT_buf,
            "v": v_buf,
            "sel": sel,
        })

    LAST_RESULT = run_bass_kernel_spmd(nc, in_maps, core_ids=list(range(NC_)))
    res = LAST_RESULT.results[0]["out"]               # (64, 4096), permuted
    out = np.empty((B, HID), np.float32)
    out[perm] = res
    return out.reshape(B, 1, HID).astype(np.float32)
